# revision 1
# baseline (speedup 1.0000x reference)
# Trainium2 Bass kernel for nn_BinaryConv (binarized VGG-ish CNN, batch 512).
#
# Strategy: pure data parallel over 8 NeuronCores (64 images each), weights
# replicated. All activations are kept as a bf16 hi/lo pair (hi = bf16(x),
# lo = bf16(x - hi)) so every conv/fc runs as 2 bf16 matmuls accumulating in
# fp32 PSUM -> ~16-bit effective mantissa. The binarized (+-1) weights are
# exact in bf16. The network's logits are ~1e12 with min top-2 relative gap
# 5e-4, so bf16 alone flips argmaxes while hi/lo reproduces the fp32
# reference's (exactly one-hot) softmax bitwise.
#
# Per layer: conv = 9 shifted-window matmuls per output-row chunk (N<=512)
# accumulated in one PSUM bank; BN+bias folded into per-channel scale/bias
# applied by the scalar engine (Relu) straight out of PSUM; 2x2 maxpool on
# the vector engine on exact fp32 values before the hi/lo split. Layer 1
# uses host-side im2col with K=54 (27 taps x hi/lo) so one matmul per chunk.

import numpy as np
import ml_dtypes

import concourse.mybir as mybir
import concourse.tile as tile
from concourse import bacc
from concourse.bass_utils import run_bass_kernel_spmd

bf16 = ml_dtypes.bfloat16
F32 = mybir.dt.float32
BF16 = mybir.dt.bfloat16
Relu = mybir.ActivationFunctionType.Relu
Exp = mybir.ActivationFunctionType.Exp
Identity = mybir.ActivationFunctionType.Identity
MULT = mybir.AluOpType.mult
SUB = mybir.AluOpType.subtract
MAX = mybir.AluOpType.max
ADD = mybir.AluOpType.add

N_CORES = 8
B = 64          # images per core
SB = 16         # L1/L2 sub-batch
N_SB = 4
EPS = 1e-5

_NC_CACHE = {}
DEBUG_TAPS = False  # adds intermediate-tensor outputs for debugging


def _split_hi_lo(nc, pool, lo_dst, y32, hi_dst):
    # hi = bf16(relu-ish copy), lo = bf16(y32 - hi). y32 is already >= 0.
    nc.scalar.activation(hi_dst, y32, Relu)
    nc.vector.scalar_tensor_tensor(lo_dst, y32, 1.0, hi_dst, op0=MULT, op1=SUB)


def build_nc():
    if "nc" in _NC_CACHE:
        return _NC_CACHE["nc"]
    nc = bacc.Bacc(None, target_bir_lowering=False, debug=False)

    # ---------------- DRAM parameters ----------------
    xi = nc.declare_dram_parameter("xi", [N_SB, 56, 30 * 30 * SB], BF16, isOutput=False)
    w1 = nc.declare_dram_parameter("w1", [56, 128], BF16, isOutput=False)
    w2 = nc.declare_dram_parameter("w2", [128, 9, 128], BF16, isOutput=False)
    w3 = nc.declare_dram_parameter("w3", [128, 9, 256], BF16, isOutput=False)
    w4 = nc.declare_dram_parameter("w4", [2, 128, 9, 256], BF16, isOutput=False)
    w5 = nc.declare_dram_parameter("w5", [2, 128, 9, 512], BF16, isOutput=False)
    w6 = nc.declare_dram_parameter("w6", [4, 128, 9, 512], BF16, isOutput=False)
    fw1 = nc.declare_dram_parameter("fw1", [4, 128, 1024], BF16, isOutput=False)
    fw2 = nc.declare_dram_parameter("fw2", [8, 128, 1024], BF16, isOutput=False)
    fw3 = nc.declare_dram_parameter("fw3", [128, 8, 10], BF16, isOutput=False)
    # consts columns: 0:s1 1:t1 2:s2 3:t2 4-5:s3 6-7:t3 8-9:s4 10-11:t4
    # 12-15:s5 16-19:t5 20-23:s6 24-27:t6 28-35:fb1 36-43:fb2 44:fb3(rows0-9)
    consts = nc.declare_dram_parameter("consts", [128, 45], F32, isOutput=False)
    ident = nc.declare_dram_parameter("ident", [16, 16], F32, isOutput=False)
    out = nc.declare_dram_parameter("out", [B, 10], F32, isOutput=True)
    taps = {}
    if DEBUG_TAPS:
        for nm, shp in [("d_l1h", [128, 30, 30, SB]), ("d_l1l", [128, 30, 30, SB]),
                        ("d_p1h", [128, 14, 14, B]), ("d_p1l", [128, 14, 14, B]),
                        ("d_l3h", [128, 12, 12, B]), ("d_p2h", [128, 5, 5, B]),
                        ("d_l5h", [128, 3, 3, B]), ("d_fth", [128, B]),
                        ("d_z1h", [128, B]), ("d_z2h", [128, B]),
                        ("d_logits", [10, B])]:
            taps[nm] = nc.declare_dram_parameter(nm, shp, F32 if nm == "d_logits" else BF16,
                                                 isOutput=True)

    with tile.TileContext(nc) as tc:
        with tc.tile_pool(name="psp", bufs=8, space="PSUM") as psp, \
             tc.tile_pool(name="p0", bufs=1) as p0:
            # ---------------- persistent tiles ----------------
            w1s = p0.tile([56, 128], BF16)
            w2s = p0.tile([128, 9, 128], BF16)
            w3s = p0.tile([128, 9, 256], BF16)
            w4s = [p0.tile([128, 9, 256], BF16, name=f"w4s{i}") for i in range(2)]
            fw1s = [p0.tile([128, 1024], BF16, name=f"fw1s{i}") for i in range(4)]
            fw3s = p0.tile([128, 8, 10], BF16)
            cs = p0.tile([128, 45], F32)
            ids = p0.tile([16, 16], F32)
            w5s = [p0.tile([128, 9, 512], BF16, name=f"w5s{i}") for i in range(2)]
            p1h = p0.tile([128, 14, 14, B], BF16)
            p1l = p0.tile([128, 14, 14, B], BF16)
            p2h = [p0.tile([128, 5, 5, B], BF16, name=f"p2h{i}") for i in range(2)]
            p2l = [p0.tile([128, 5, 5, B], BF16, name=f"p2l{i}") for i in range(2)]
            fth = [p0.tile([128, B], BF16, name=f"fth{i}") for i in range(4)]
            ftl = [p0.tile([128, B], BF16, name=f"ftl{i}") for i in range(4)]
            z1h = [p0.tile([128, B], BF16, name=f"z1h{i}") for i in range(8)]
            z1l = [p0.tile([128, B], BF16, name=f"z1l{i}") for i in range(8)]
            z2h = [p0.tile([128, B], BF16, name=f"z2h{i}") for i in range(8)]
            z2l = [p0.tile([128, B], BF16, name=f"z2l{i}") for i in range(8)]

            # only what L1/L2 need immediately; bulk weights stream after the
            # first im2col DMAs so they don't block kernel start
            nc.sync.dma_start(out=w1s[:], in_=w1[:])
            nc.sync.dma_start(out=w2s[:], in_=w2[:])
            nc.sync.dma_start(out=cs[:], in_=consts[:])
            nc.sync.dma_start(out=ids[:], in_=ident[:])

            def load_bulk_weights():
                nc.sync.dma_start(out=w3s[:], in_=w3[:])
                for i in range(2):
                    nc.sync.dma_start(out=w4s[i][:], in_=w4[i])
                for i in range(4):
                    nc.sync.dma_start(out=fw1s[i][:], in_=fw1[i])
                nc.sync.dma_start(out=fw3s[:], in_=fw3[:])
                for i in range(2):
                    nc.sync.dma_start(out=w5s[i][:], in_=w5[i])

            def col(j):
                return cs[:, j:j + 1]

            # =============== phase A: L1, L2, pool1 (per sub-batch) ===============
            # L1 row r is emitted, then L2 output row r-4 (which only needs L1
            # rows r-4..r-2) — the single-matmul L1 chunks' ACT drain hides
            # under L2's 18-matmul chunks instead of serializing before them;
            # the 2-row extra skew keeps L2's l1l operand (ACT->DVE chain)
            # ready before the PE reaches it.
            with tc.tile_pool(name="pA", bufs=1) as pA:
                # single l1 tile pair for all sub-batches: subtile (per-row)
                # dependencies let L1(sb+1) rows start while L2(sb) tail rows
                # still read the old rows, so the skew pipelines ACROSS
                # sub-batch boundaries instead of draining at each one.
                l1h = pA.tile([128, 30, 30, SB], BF16, tag="l1h")
                l1l = pA.tile([128, 30, 30, SB], BF16, tag="l1l")
                prev_row = [None]

                def l1_row(sb, r):
                    ic = pA.tile([56, 30, SB], BF16, tag="ic", bufs=6, name=f"ic_{sb}_{r}")
                    nc.sync.dma_start(
                        out=ic[:], in_=xi[sb, :, r * 30 * SB:(r + 1) * 30 * SB])
                    ps = psp.tile([128, 30, SB], F32, tag="ps", name=f"ps1_{sb}_{r}")
                    nc.tensor.matmul(ps[:], w1s[:], ic[:], start=True, stop=True)
                    y32 = pA.tile([128, 30, SB], F32, tag="y32", bufs=5, name=f"y1_{sb}_{r}")
                    nc.vector.tensor_scalar(y32[:], ps[:], col(0), 0.0, op0=MULT, op1=MAX)
                    nc.scalar.activation(l1h[:, r], ps[:], Relu, scale=col(0))
                    nc.vector.scalar_tensor_tensor(
                        l1l[:, r], y32[:], 1.0, l1h[:, r], op0=MULT, op1=SUB)

                def l2_row(sb, q):
                    bsl = slice(sb * SB, (sb + 1) * SB)
                    ps = psp.tile([128, 28, SB], F32, tag="ps", name=f"ps2_{sb}_{q}")
                    first = True
                    for part in (l1h, l1l):
                        for dh in range(3):
                            for dw in range(3):
                                nc.tensor.matmul(
                                    ps[:], w2s[:, dh * 3 + dw, :],
                                    part[:, q + dh, dw:dw + 28, :],
                                    start=first, stop=(part is l1l and dh == 2 and dw == 2))
                                first = False
                    y32 = pA.tile([128, 28, SB], F32, tag="y32", bufs=5,
                                  name=f"y2_{sb}_{q}")
                    nc.scalar.activation(y32[:], ps[:], Relu, bias=col(3), scale=col(2))
                    if q % 2 == 0:
                        prev_row[0] = y32
                        return
                    p = q // 2
                    rm = pA.tile([128, 28, SB], F32, tag="rm", bufs=2, name=f"rm_{sb}_{p}")
                    nc.vector.tensor_tensor(rm[:], prev_row[0][:], y32[:], op=MAX)
                    rmv = rm[:].rearrange("p (w two) b -> p w two b", two=2)
                    pw = pA.tile([128, 14, SB], F32, tag="pw", bufs=2, name=f"pw_{sb}_{p}")
                    nc.vector.tensor_tensor(pw[:], rmv[:, :, 0, :], rmv[:, :, 1, :], op=MAX)
                    nc.scalar.activation(p1h[:, p, :, bsl], pw[:], Relu)
                    nc.vector.scalar_tensor_tensor(
                        p1l[:, p, :, bsl], pw[:], 1.0, p1h[:, p, :, bsl],
                        op0=MULT, op1=SUB)

                SKEW = 4
                for gi in range(N_SB * 30 + SKEW):
                    if gi < N_SB * 30:
                        sb1, r = divmod(gi, 30)
                        l1_row(sb1, r)
                    if gi == 12:
                        load_bulk_weights()
                    gq = gi - SKEW
                    if gq >= 0:
                        sb2, q = divmod(gq, 30)
                        if q < 28:
                            l2_row(sb2, q)
                if DEBUG_TAPS:
                    nc.sync.dma_start(out=taps["d_l1h"][:], in_=l1h[:])
                    nc.sync.dma_start(out=taps["d_l1l"][:], in_=l1l[:])

            if DEBUG_TAPS:
                nc.sync.dma_start(out=taps["d_p1h"][:], in_=p1h[:])
                nc.sync.dma_start(out=taps["d_p1l"][:], in_=p1l[:])
            # =============== phase B: L3, L4, pool2 (full batch) ===============
            with tc.tile_pool(name="pB", bufs=1) as pB:
                l3h = [pB.tile([128, 12, 12, B], BF16, name=f"l3h{i}") for i in range(2)]
                l3l = [pB.tile([128, 12, 12, B], BF16, name=f"l3l{i}") for i in range(2)]
                # ---- L3 ----
                for cog in range(2):
                    wsl = slice(cog * 128, (cog + 1) * 128)
                    for r in range(12):
                        for bh in range(2):
                            bsl = slice(bh * 32, (bh + 1) * 32)
                            ps = psp.tile([128, 12, 32], F32, tag="ps")
                            first = True
                            for part in (p1h, p1l):
                                for dh in range(3):
                                    for dw in range(3):
                                        nc.tensor.matmul(
                                            ps[:], w3s[:, dh * 3 + dw, wsl],
                                            part[:, r + dh, dw:dw + 12, bsl],
                                            start=first,
                                            stop=(part is p1l and dh == 2 and dw == 2))
                                        first = False
                            y32 = pB.tile([128, 12, 32], F32, tag="y32b", bufs=4,
                                          name=f"y3_{cog}_{r}_{bh}")
                            nc.scalar.activation(y32[:], ps[:], Relu,
                                                 bias=col(6 + cog), scale=col(4 + cog))
                            nc.scalar.activation(l3h[cog][:, r, :, bsl], ps[:], Relu,
                                                 bias=col(6 + cog), scale=col(4 + cog))
                            nc.vector.scalar_tensor_tensor(
                                l3l[cog][:, r, :, bsl], y32[:], 1.0,
                                l3h[cog][:, r, :, bsl], op0=MULT, op1=SUB)
                if DEBUG_TAPS:
                    nc.sync.dma_start(out=taps["d_l3h"][:], in_=l3h[0][:])
                # ---- L4 + pool2 ----
                for cog in range(2):
                    wsl = slice(cog * 128, (cog + 1) * 128)
                    for bh in range(2):
                        bsl = slice(bh * 32, (bh + 1) * 32)
                        for p in range(5):
                            rows = []
                            for rr in range(2):
                                r = 2 * p + rr
                                ps = psp.tile([128, 10, 32], F32, tag="ps")
                                first = True
                                for cb in range(2):
                                    for part in (l3h, l3l):
                                        for dh in range(3):
                                            for dw in range(3):
                                                nc.tensor.matmul(
                                                    ps[:], w4s[cb][:, dh * 3 + dw, wsl],
                                                    part[cb][:, r + dh, dw:dw + 10, bsl],
                                                    start=first,
                                                    stop=(cb == 1 and part is l3l
                                                          and dh == 2 and dw == 2))
                                                first = False
                                y32 = pB.tile([128, 10, 32], F32, tag="y32b", bufs=4,
                                              name=f"y4_{cog}_{bh}_{p}_{rr}")
                                nc.scalar.activation(y32[:], ps[:], Relu,
                                                     bias=col(10 + cog), scale=col(8 + cog))
                                rows.append(y32)
                            rm = pB.tile([128, 10, 32], F32, tag="rm4", bufs=2)
                            nc.vector.tensor_tensor(rm[:], rows[0][:], rows[1][:], op=MAX)
                            rmv = rm[:].rearrange("p (w two) b -> p w two b", two=2)
                            pw = pB.tile([128, 5, 32], F32, tag="pw4", bufs=2)
                            nc.vector.tensor_tensor(pw[:], rmv[:, :, 0, :], rmv[:, :, 1, :],
                                                    op=MAX)
                            nc.scalar.activation(p2h[cog][:, p, :, bsl], pw[:], Relu)
                            nc.vector.scalar_tensor_tensor(
                                p2l[cog][:, p, :, bsl], pw[:], 1.0,
                                p2h[cog][:, p, :, bsl], op0=MULT, op1=SUB)

            # =============== phase C: L5, L6, FC, softmax ===============
            with tc.tile_pool(name="pC", bufs=1) as pC:
                w6s = [pC.tile([128, 9, 512], BF16, name=f"w6s{i}") for i in range(4)]
                l5h = [pC.tile([128, 3, 3, B], BF16, name=f"l5h{i}") for i in range(4)]
                l5l = [pC.tile([128, 3, 3, B], BF16, name=f"l5l{i}") for i in range(4)]
                fw2s = [pC.tile([128, 1024], BF16, name=f"fw2s{i}") for i in range(8)]
                for i in range(4):
                    nc.sync.dma_start(out=w6s[i][:], in_=w6[i])
                for i in range(8):
                    nc.sync.dma_start(out=fw2s[i][:], in_=fw2[i])

                if DEBUG_TAPS:
                    nc.sync.dma_start(out=taps["d_p2h"][:], in_=p2h[0][:])
                # ---- L5 ----
                for cog in range(4):
                    wsl = slice(cog * 128, (cog + 1) * 128)
                    for (h0, nh) in ((0, 2), (2, 1)):
                        ps = psp.tile([128, nh, 3, B], F32, tag="ps")
                        first = True
                        for cb in range(2):
                            for part in (p2h, p2l):
                                for dh in range(3):
                                    for dw in range(3):
                                        nc.tensor.matmul(
                                            ps[:], w5s[cb][:, dh * 3 + dw, wsl],
                                            part[cb][:, h0 + dh:h0 + dh + nh, dw:dw + 3, :],
                                            start=first,
                                            stop=(cb == 1 and part is p2l
                                                  and dh == 2 and dw == 2))
                                        first = False
                        y32 = pC.tile([128, nh, 3, B], F32, tag="y32c", bufs=6,
                                      name=f"y5_{cog}_{h0}")
                        nc.scalar.activation(y32[:], ps[:], Relu,
                                             bias=col(16 + cog), scale=col(12 + cog))
                        nc.vector.tensor_copy(l5h[cog][:, h0:h0 + nh], y32[:])
                        nc.vector.scalar_tensor_tensor(
                            l5l[cog][:, h0:h0 + nh], y32[:], 1.0,
                            l5h[cog][:, h0:h0 + nh], op0=MULT, op1=SUB)

                if DEBUG_TAPS:
                    nc.sync.dma_start(out=taps["d_l5h"][:], in_=l5h[0][:])
                # ---- L6 (3x3 conv on 3x3 input == dense over (ci, s)) ----
                for cog in range(4):
                    wsl = slice(cog * 128, (cog + 1) * 128)
                    ps = psp.tile([128, B], F32, tag="ps")
                    first = True
                    for cb in range(4):
                        for part in (l5h, l5l):
                            pv = part[cb][:].rearrange("p h w b -> p (h w) b")
                            for s in range(9):
                                nc.tensor.matmul(
                                    ps[:], w6s[cb][:, s, wsl], pv[:, s, :],
                                    start=first,
                                    stop=(cb == 3 and part is l5l and s == 8))
                                first = False
                    y32 = pC.tile([128, B], F32, tag="y32c", bufs=6, name=f"y6_{cog}")
                    nc.scalar.activation(y32[:], ps[:], Relu,
                                         bias=col(24 + cog), scale=col(20 + cog))
                    nc.vector.tensor_copy(fth[cog][:], y32[:])
                    nc.vector.scalar_tensor_tensor(
                        ftl[cog][:], y32[:], 1.0, fth[cog][:], op0=MULT, op1=SUB)

                if DEBUG_TAPS:
                    nc.sync.dma_start(out=taps["d_fth"][:], in_=fth[0][:])
                # ---- FC1 ----
                for cog in range(8):
                    wsl = slice(cog * 128, (cog + 1) * 128)
                    ps = psp.tile([128, B], F32, tag="ps")
                    first = True
                    for kb in range(4):
                        for part in (fth, ftl):
                            nc.tensor.matmul(ps[:], fw1s[kb][:, wsl], part[kb][:],
                                             start=first,
                                             stop=(kb == 3 and part is ftl))
                            first = False
                    y32 = pC.tile([128, B], F32, tag="y32c", bufs=6, name=f"yf1_{cog}")
                    nc.vector.tensor_scalar(y32[:], ps[:], col(28 + cog), 0.0, op0=ADD, op1=MAX)
                    nc.scalar.activation(z1h[cog][:], ps[:], Relu, bias=col(28 + cog), scale=1.0)
                    nc.vector.scalar_tensor_tensor(
                        z1l[cog][:], y32[:], 1.0, z1h[cog][:], op0=MULT, op1=SUB)

                if DEBUG_TAPS:
                    nc.sync.dma_start(out=taps["d_z1h"][:], in_=z1h[0][:])
                # ---- FC2 ----
                for cog in range(8):
                    wsl = slice(cog * 128, (cog + 1) * 128)
                    ps = psp.tile([128, B], F32, tag="ps")
                    first = True
                    for kb in range(8):
                        for part in (z1h, z1l):
                            nc.tensor.matmul(ps[:], fw2s[kb][:, wsl], part[kb][:],
                                             start=first,
                                             stop=(kb == 7 and part is z1l))
                            first = False
                    y32 = pC.tile([128, B], F32, tag="y32c", bufs=6, name=f"yf2_{cog}")
                    nc.vector.tensor_scalar(y32[:], ps[:], col(36 + cog), 0.0, op0=ADD, op1=MAX)
                    nc.scalar.activation(z2h[cog][:], ps[:], Relu, bias=col(36 + cog), scale=1.0)
                    nc.vector.scalar_tensor_tensor(
                        z2l[cog][:], y32[:], 1.0, z2h[cog][:], op0=MULT, op1=SUB)

                if DEBUG_TAPS:
                    nc.sync.dma_start(out=taps["d_z2h"][:], in_=z2h[0][:])
                # ---- FC3 + softmax ----
                ps = psp.tile([10, B], F32, tag="ps")
                first = True
                for kb in range(8):
                    for part in (z2h, z2l):
                        nc.tensor.matmul(ps[:], fw3s[:, kb, :], part[kb][:],
                                         start=first, stop=(kb == 7 and part is z2l))
                        first = False
                logits = pC.tile([10, B], F32)
                nc.vector.tensor_scalar_add(logits[:], ps[:], cs[0:10, 44:45])
                if DEBUG_TAPS:
                    nc.sync.dma_start(out=taps["d_logits"][:], in_=logits[:])
                pst = psp.tile([B, 10], F32, tag="ps")
                nc.tensor.transpose(pst[:], logits[:], ids[0:10, 0:10])
                nm = pC.tile([B, 1], F32)
                nc.vector.tensor_reduce(out=nm[:], in_=pst[:], op=MAX,
                                        axis=mybir.AxisListType.X, negate=True)
                ex = pC.tile([B, 10], F32)
                sm = pC.tile([B, 1], F32)
                nc.scalar.activation(ex[:], pst[:], Exp, bias=nm[:], scale=1.0,
                                     accum_out=sm[:])
                rc = pC.tile([B, 1], F32)
                nc.vector.reciprocal(rc[:], sm[:])
                so = pC.tile([B, 10], F32)
                nc.vector.tensor_scalar_mul(so[:], ex[:], rc[:])
                nc.sync.dma_start(out=out[:], in_=so[:])

    nc.compile()
    _NC_CACHE["nc"] = nc
    return nc


# ---------------- host-side data prep ----------------

def _fold_bn(b, g, be, m, v):
    inv = (g / np.sqrt(v + EPS)).astype(np.float32)
    return inv, ((b - m) * inv + be).astype(np.float32)


def _conv_w(w):
    # [co, ci, kh, kw] +-1 -> [ci, kh*3+kw, co] bf16 (split over 128-blocks of ci outside)
    return np.ascontiguousarray(np.sign(w).transpose(1, 2, 3, 0).reshape(
        w.shape[1], 9, w.shape[0])).astype(bf16)


def _prep_shared(inputs):
    d = {}
    w1c = _conv_w(inputs["w1"]).reshape(27, 128)
    s1f, t1f = _fold_bn(inputs["b1"], inputs["g1"], inputs["be1"],
                        inputs["m1"], inputs["v1"])
    bias_row = (t1f / s1f).astype(np.float32)
    bh = bias_row.astype(bf16)
    bl = (bias_row - bh.astype(np.float32)).astype(bf16)
    d["w1"] = np.vstack([w1c, w1c, bh[None, :], bl[None, :]])
    d["w2"] = _conv_w(inputs["w2"])
    d["w3"] = _conv_w(inputs["w3"])
    d["w4"] = np.ascontiguousarray(_conv_w(inputs["w4"]).reshape(2, 128, 9, 256))
    d["w5"] = np.ascontiguousarray(_conv_w(inputs["w5"]).reshape(2, 128, 9, 512))
    d["w6"] = np.ascontiguousarray(_conv_w(inputs["w6"]).reshape(4, 128, 9, 512))
    for nm, k in (("fw1", 4), ("fw2", 8)):
        w = np.sign(inputs[nm]).T.astype(bf16)  # [K, co]
        d[nm] = np.ascontiguousarray(w.reshape(k, 128, w.shape[1]))
    w = np.sign(inputs["fw3"]).T.astype(bf16)  # [1024, 10]
    d["fw3"] = np.ascontiguousarray(w.reshape(8, 128, 10).transpose(1, 0, 2))
    consts = np.zeros((128, 45), np.float32)
    coff = [(1, 0, 1), (2, 2, 3), (3, 4, 6), (4, 8, 10), (5, 12, 16), (6, 20, 24)]
    for li, so, to in coff:
        s, t = _fold_bn(inputs[f"b{li}"], inputs[f"g{li}"], inputs[f"be{li}"],
                        inputs[f"m{li}"], inputs[f"v{li}"])
        nb = len(s) // 128
        for j in range(nb):
            consts[:, so + j] = s[j * 128:(j + 1) * 128]
            consts[:, to + j] = t[j * 128:(j + 1) * 128]
    for j in range(8):
        consts[:, 28 + j] = inputs["fb1"][j * 128:(j + 1) * 128]
        consts[:, 36 + j] = inputs["fb2"][j * 128:(j + 1) * 128]
    consts[0:10, 44] = inputs["fb3"]
    d["consts"] = consts
    d["ident"] = np.eye(16, dtype=np.float32)
    return d


def _prep_x(xc):
    # xc [B, 3, 32, 32] f32 -> im2col [N_SB, 54, 30*30*SB] bf16 (hi rows 0-26, lo 27-53)
    x32 = xc.astype(np.float32)
    hi = x32.astype(bf16)
    lo = (x32 - hi.astype(np.float32)).astype(bf16)
    parts = []
    for p in (hi, lo):
        win = np.lib.stride_tricks.sliding_window_view(p, (3, 3), axis=(2, 3))
        # win [B, ci, r, w, dh, dw] -> [ci, dh, dw, r, w, B]
        arr = win.transpose(1, 4, 5, 2, 3, 0).reshape(27, 30, 30, B)
        parts.append(arr)
    ones = np.ones((2, 30, 30, B), bf16)
    full = np.concatenate(parts + [ones], axis=0)  # [56, 30, 30, B]
    full = full.reshape(56, 30, 30, N_SB, SB).transpose(3, 0, 1, 2, 4)
    return np.ascontiguousarray(full).reshape(N_SB, 56, 30 * 30 * SB)


def make_in_maps(inputs):
    shared = _prep_shared(inputs)
    x = np.asarray(inputs["x"])
    in_maps = []
    for c in range(N_CORES):
        m = dict(shared)
        m["xi"] = _prep_x(x[c * B:(c + 1) * B])
        in_maps.append(m)
    return in_maps


def kernel(**inputs):
    nc = build_nc()
    in_maps = make_in_maps(inputs)
    res = run_bass_kernel_spmd(nc, in_maps, list(range(N_CORES)))
    return np.concatenate([res.results[c]["out"] for c in range(N_CORES)], axis=0)



# revision 12
# speedup vs baseline: 1.1728x; 1.1728x over previous
# Trainium2 Bass kernel for nn_BinaryConv (binarized VGG-ish CNN, batch 512).
#
# Strategy: pure data parallel over 8 NeuronCores (64 images each), weights
# replicated. Precision: every layer's activations are decomposed into terms
# whose matmuls accumulate exactly in fp32 PSUM:
#   - L2 input (L1 output): bf16 hi term `h` + e5m2 residual pair (t2,t3)
#     -> 9 bf16 matmuls + 9 fp8 DoubleRow matmuls per output row chunk.
#   - L3..L6 / FC inputs: 4-term fp8 split: e4m3 pair (t0,t1) + e5m2 pair
#     (t2,t3), consumed exclusively with fp8 DoubleRow matmuls (2 k-tiles
#     per instruction at 0.5 cycles/row = 4x bf16 throughput per k-tile).
# Effective mantissa ~16 bits, matching the fp32 reference closely enough
# to reproduce its (exactly one-hot) softmax output.
#
# DoubleRow pairing: vertical tap pairs (dh=0,1 at fixed dw) ride adjacent
# h-slices of the stored activation tile; the leftover dh=2 taps pair the
# two split terms of one tap (weights duplicated host-side for those rows).
# Per-layer power-of-2 output scales keep e4m3 terms below its 240 max.
# BN+bias are folded into per-channel scale/bias applied out of PSUM.

import numpy as np
import ml_dtypes

import concourse.mybir as mybir
import concourse.tile as tile
from concourse import bacc
from concourse.bass_utils import run_bass_kernel_spmd

bf16 = ml_dtypes.bfloat16
e4m3 = ml_dtypes.float8_e4m3
e5m2 = ml_dtypes.float8_e5m2
F32 = mybir.dt.float32
BF16 = mybir.dt.bfloat16
F8E4 = mybir.dt.float8e4
F8E5 = mybir.dt.float8e5
Relu = mybir.ActivationFunctionType.Relu
Copy = mybir.ActivationFunctionType.Copy
Exp = mybir.ActivationFunctionType.Exp
MULT = mybir.AluOpType.mult
SUB = mybir.AluOpType.subtract
MAX = mybir.AluOpType.max
ADD = mybir.AluOpType.add
DR = mybir.MatmulPerfMode.DoubleRow

N_CORES = 8
B = 64          # images per core
SB = 16         # L1/L2 sub-batch
N_SB = 4
EPS = 1e-5

# Per-layer power-of-2 output scales (stored activation = SIG[l] * true).
# Chosen so each scaled tensor's max stays well under e4m3's 240 limit.
SIG = [2.0 ** e for e in (1, -4, -9, -14, -19, -25, -28, -33)]
# y1, p1, l3, l4(p2), l5, l6, fc1, fc2 output scales (validated in proto)

_NC_CACHE = {}


def _c(v):
    return float(np.float32(v))


def build_nc():
    if "nc" in _NC_CACHE:
        return _NC_CACHE["nc"]
    nc = bacc.Bacc(None, target_bir_lowering=False, debug=False)

    # ---------------- DRAM parameters ----------------
    xi = nc.declare_dram_parameter("xi", [N_SB, 56, 30 * 30 * SB], BF16, isOutput=False)
    w1 = nc.declare_dram_parameter("w1", [56, 128], BF16, isOutput=False)
    w2h = nc.declare_dram_parameter("w2h", [128, 9, 128], BF16, isOutput=False)
    w2e = nc.declare_dram_parameter("w2e", [128, 6, 2, 128], F8E5, isOutput=False)
    w3a = nc.declare_dram_parameter("w3a", [128, 2, 6, 2, 128], F8E4, isOutput=False)
    w3b = nc.declare_dram_parameter("w3b", [128, 2, 6, 2, 128], F8E5, isOutput=False)
    w4a = nc.declare_dram_parameter("w4a", [2, 128, 2, 6, 2, 128], F8E4, isOutput=False)
    w4b = nc.declare_dram_parameter("w4b", [2, 128, 2, 6, 2, 128], F8E5, isOutput=False)
    w5a = nc.declare_dram_parameter("w5a", [2, 128, 4, 6, 2, 128], F8E4, isOutput=False)
    w5b = nc.declare_dram_parameter("w5b", [2, 128, 4, 6, 2, 128], F8E5, isOutput=False)
    w6a = nc.declare_dram_parameter("w6a", [4, 128, 4, 5, 2, 128], F8E4, isOutput=False)
    w6b = nc.declare_dram_parameter("w6b", [4, 128, 4, 5, 2, 128], F8E5, isOutput=False)
    fw1a = nc.declare_dram_parameter("fw1a", [128, 8, 2, 2, 128], F8E4, isOutput=False)
    fw1b = nc.declare_dram_parameter("fw1b", [128, 8, 2, 2, 128], F8E5, isOutput=False)
    fw2a = nc.declare_dram_parameter("fw2a", [128, 8, 4, 2, 128], F8E4, isOutput=False)
    fw2b = nc.declare_dram_parameter("fw2b", [128, 8, 4, 2, 128], F8E5, isOutput=False)
    fw3a = nc.declare_dram_parameter("fw3a", [128, 4, 2, 16], F8E4, isOutput=False)
    fw3b = nc.declare_dram_parameter("fw3b", [128, 4, 2, 16], F8E5, isOutput=False)
    # consts columns: 0:s1 1:s2 2:t2 3-4:s3 5-6:t3 7-8:s4 9-10:t4
    # 11-14:s5 15-18:t5 19-22:s6 23-26:t6 27-34:fb1 35-42:fb2 43:fb3(rows0-9)
    consts = nc.declare_dram_parameter("consts", [128, 44], F32, isOutput=False)
    ident = nc.declare_dram_parameter("ident", [16, 16], F32, isOutput=False)
    out = nc.declare_dram_parameter("out", [B, 10], F32, isOutput=True)

    inv_f2 = _c(1.0 / SIG[7])

    with tile.TileContext(nc) as tc:
        with tc.tile_pool(name="psp", bufs=8, space="PSUM") as psp, \
             tc.tile_pool(name="p0", bufs=1) as p0:
            # ---------------- persistent tiles ----------------
            w1s = p0.tile([56, 128], BF16)
            w2hs = p0.tile([128, 9, 128], BF16)
            w2es = p0.tile([128, 6, 2, 128], F8E5)
            w3as = p0.tile([128, 2, 6, 2, 128], F8E4)
            w3bs = p0.tile([128, 2, 6, 2, 128], F8E5)
            w4as = [p0.tile([128, 2, 6, 2, 128], F8E4, name=f"w4as{i}") for i in range(2)]
            w4bs = [p0.tile([128, 2, 6, 2, 128], F8E5, name=f"w4bs{i}") for i in range(2)]
            w5as = [p0.tile([128, 4, 6, 2, 128], F8E4, name=f"w5as{i}") for i in range(2)]
            w5bs = [p0.tile([128, 4, 6, 2, 128], F8E5, name=f"w5bs{i}") for i in range(2)]
            fw1as = p0.tile([128, 8, 2, 2, 128], F8E4)
            fw1bs = p0.tile([128, 8, 2, 2, 128], F8E5)
            cs = p0.tile([128, 44], F32)
            ids = p0.tile([16, 16], F32)
            p1a = [p0.tile([128, 2, 14, 14, 32], F8E4, name=f"p1a{i}") for i in range(2)]
            p1b = [p0.tile([128, 2, 14, 14, 32], F8E5, name=f"p1b{i}") for i in range(2)]
            p2a = [p0.tile([128, 2, 5, 5, B], F8E4, name=f"p2a{i}") for i in range(2)]
            p2b = [p0.tile([128, 2, 5, 5, B], F8E5, name=f"p2b{i}") for i in range(2)]

            nc.sync.dma_start(out=w1s[:], in_=w1[:])
            nc.sync.dma_start(out=w2hs[:], in_=w2h[:])
            nc.sync.dma_start(out=w2es[:], in_=w2e[:])
            nc.sync.dma_start(out=cs[:], in_=consts[:])
            nc.sync.dma_start(out=ids[:], in_=ident[:])

            def load_phaseb_weights():
                nc.sync.dma_start(out=w3as[:], in_=w3a[:])
                nc.sync.dma_start(out=w3bs[:], in_=w3b[:])
                for i in range(2):
                    nc.sync.dma_start(out=w4as[i][:], in_=w4a[i])
                    nc.sync.dma_start(out=w4bs[i][:], in_=w4b[i])

            def load_phasec_weights():
                for i in range(2):
                    nc.sync.dma_start(out=w5as[i][:], in_=w5a[i])
                    nc.sync.dma_start(out=w5bs[i][:], in_=w5b[i])
                nc.sync.dma_start(out=fw1as[:], in_=fw1a[:])
                nc.sync.dma_start(out=fw1bs[:], in_=fw1b[:])

            def col(j):
                return cs[:, j:j + 1]

            class MM:
                """start/stop bookkeeping for one PSUM accumulation group."""
                def __init__(self, ps, n):
                    self.ps, self.n, self.i = ps, n, 0

                def mm(self, lhsT, rhs, dr=False):
                    nc.tensor.matmul(self.ps[:], lhsT, rhs,
                                     start=(self.i == 0), stop=(self.i == self.n - 1),
                                     perf_mode=DR if dr else None)
                    self.i += 1

            def dr9(mmo, wa_, pt, hsl, wlen):
                """9 DoubleRow matmuls covering 2 terms x 9 taps of one dtype.

                pt: [128, 2, H, W, Bd] tile; hsl: first input row; window w
                length wlen. Weights wa_: [128, 6, 2, M] pair-contiguous
                (blocks 0-2: vertical dh01 pairs per dwi; 3-5: dh2 dups)."""
                for t in range(2):
                    for dwi in range(3):
                        mmo.mm(wa_[:, dwi],
                               pt[:, t, hsl:hsl + 2, dwi:dwi + wlen, :], dr=True)
                for dwi in range(3):
                    mmo.mm(wa_[:, 3 + dwi],
                           pt[:, 0:2, hsl + 2, dwi:dwi + wlen, :], dr=True)

            def split4(pool, y32, ta0, ta1, tb0, tb1, tag, bufs=3):
                """4-term split of relu'd scaled fp32 y32 into e4m3 pair
                (ta0, ta1) + e5m2 pair (tb0, tb1).
                Engines: pure casts ride gpsimd software-DGE casting DMAs
                (Pool engine posts descriptors, transfer on idle DMA engines);
                subtracts on DVE; one cast each on ACT (and optionally t0)."""
                shp = [y32.tensor.shape[0]] + list(y32.tensor.shape[1:])
                nc.scalar.activation(ta0, y32[:], Relu)
                r1 = pool.tile(shp, F32, tag=f"{tag}_r1", bufs=bufs)
                nc.vector.scalar_tensor_tensor(r1[:], y32[:], 1.0, ta0,
                                               op0=MULT, op1=SUB)
                nc.gpsimd.dma_start(out=ta1, in_=r1[:])
                r2 = pool.tile(shp, F32, tag=f"{tag}_r2", bufs=bufs)
                nc.vector.scalar_tensor_tensor(r2[:], r1[:], 1.0, ta1,
                                               op0=MULT, op1=SUB)
                nc.scalar.activation(tb0, r2[:], Copy)
                nc.vector.scalar_tensor_tensor(tb1, r2[:], 1.0, tb0,
                                               op0=MULT, op1=SUB)

            # =============== phase A: L1, L2, pool1 (per sub-batch) ===============
            with tc.tile_pool(name="pA", bufs=1) as pA:
                l1h = pA.tile([128, 30, 30, SB], BF16, tag="l1h")
                l1e = pA.tile([128, 2, 30, 30, SB], F8E5, tag="l1e")
                prev_row = [None]

                def l1_row(sb, r):
                    ic = pA.tile([56, 30, SB], BF16, tag="ic", bufs=5, name=f"ic_{sb}_{r}")
                    nc.sync.dma_start(
                        out=ic[:], in_=xi[sb, :, r * 30 * SB:(r + 1) * 30 * SB])
                    ps = psp.tile([128, 30, SB], F32, tag="ps", name=f"ps1_{sb}_{r}")
                    nc.tensor.matmul(ps[:], w1s[:], ic[:], start=True, stop=True)
                    y32 = pA.tile([128, 30, SB], F32, tag="y32", bufs=3, name=f"y1_{sb}_{r}")
                    nc.scalar.activation(y32[:], ps[:], Relu, scale=col(0))
                    nc.gpsimd.dma_start(out=l1h[:, r], in_=y32[:])
                    l32 = pA.tile([128, 30, SB], F32, tag="l32", bufs=3, name=f"lw_{sb}_{r}")
                    nc.vector.scalar_tensor_tensor(l32[:], y32[:], 1.0, l1h[:, r],
                                                   op0=MULT, op1=SUB)
                    nc.scalar.activation(l1e[:, 0, r], l32[:], Copy)
                    nc.vector.scalar_tensor_tensor(l1e[:, 1, r], l32[:], 1.0,
                                                   l1e[:, 0, r], op0=MULT, op1=SUB)

                def l2_row(sb, q):
                    bsl = slice(sb * SB, (sb + 1) * SB)
                    ps = psp.tile([128, 28, SB], F32, tag="ps", name=f"ps2_{sb}_{q}")
                    mmo = MM(ps, 18)
                    for dh in range(3):
                        for dw in range(3):
                            mmo.mm(w2hs[:, dh * 3 + dw, :],
                                   l1h[:, q + dh, dw:dw + 28, :])
                    dr9(mmo, w2es, l1e, q, 28)
                    y32 = pA.tile([128, 28, SB], F32, tag="y2", bufs=4,
                                  name=f"y2_{sb}_{q}")
                    nc.scalar.activation(y32[:], ps[:], Relu, bias=col(2), scale=col(1))
                    if q % 2 == 0:
                        prev_row[0] = y32
                        return
                    p = q // 2
                    rm = pA.tile([128, 28, SB], F32, tag="rm", bufs=2, name=f"rm_{sb}_{p}")
                    nc.vector.tensor_tensor(rm[:], prev_row[0][:], y32[:], op=MAX)
                    rmv = rm[:].rearrange("p (w two) b -> p w two b", two=2)
                    pw = pA.tile([128, 14, SB], F32, tag="pw", bufs=2, name=f"pw_{sb}_{p}")
                    nc.vector.tensor_tensor(pw[:], rmv[:, :, 0, :], rmv[:, :, 1, :], op=MAX)
                    hb, hsl = sb // 2, slice((sb % 2) * SB, (sb % 2 + 1) * SB)
                    split4(pA, pw,
                           p1a[hb][:, 0, p, :, hsl], p1a[hb][:, 1, p, :, hsl],
                           p1b[hb][:, 0, p, :, hsl], p1b[hb][:, 1, p, :, hsl],
                           tag="p1s", bufs=2)

                SKEW = 4
                for gi in range(N_SB * 30 + SKEW):
                    if gi < N_SB * 30:
                        sb1, r = divmod(gi, 30)
                        l1_row(sb1, r)
                    if gi == 12:
                        load_phaseb_weights()
                    gq = gi - SKEW
                    if gq >= 0:
                        sb2, q = divmod(gq, 30)
                        if q < 28:
                            l2_row(sb2, q)

            # ====== phase B: L3, L4, pool2 (split over batch halves) ======
            with tc.tile_pool(name="pB", bufs=1) as pB:
                for bh in range(2):
                    bsl = slice(bh * 32, (bh + 1) * 32)
                    l3a = [pB.tile([128, 2, 12, 12, 32], F8E4, tag=f"l3a{i}",
                                   name=f"l3a{i}_{bh}") for i in range(2)]
                    l3b = [pB.tile([128, 2, 12, 12, 32], F8E5, tag=f"l3b{i}",
                                   name=f"l3b{i}_{bh}") for i in range(2)]
                    # ---- L3 ----
                    for cog in range(2):
                        for r in range(12):
                            ps = psp.tile([128, 12, 32], F32, tag="ps")
                            mmo = MM(ps, 18)
                            dr9(mmo, w3as[:, cog], p1a[bh], r, 12)
                            dr9(mmo, w3bs[:, cog], p1b[bh], r, 12)
                            y32 = pB.tile([128, 12, 32], F32, tag="y3b", bufs=3,
                                          name=f"y3_{cog}_{r}_{bh}")
                            nc.scalar.activation(y32[:], ps[:], Relu,
                                                 bias=col(5 + cog), scale=col(3 + cog))
                            split4(pB, y32,
                                   l3a[cog][:, 0, r], l3a[cog][:, 1, r],
                                   l3b[cog][:, 0, r], l3b[cog][:, 1, r],
                                   tag="l3s", bufs=2)
                            if bh == 0 and cog == 1 and r == 5:
                                load_phasec_weights()
                    # ---- L4 + pool2 ----
                    for cog in range(2):
                        for p in range(5):
                            rows = []
                            for rr in range(2):
                                r = 2 * p + rr
                                ps = psp.tile([128, 10, 32], F32, tag="ps")
                                mmo = MM(ps, 36)
                                for cb in range(2):
                                    dr9(mmo, w4as[cb][:, cog], l3a[cb], r, 10)
                                    dr9(mmo, w4bs[cb][:, cog], l3b[cb], r, 10)
                                y32 = pB.tile([128, 10, 32], F32, tag="y4b", bufs=3,
                                              name=f"y4_{cog}_{bh}_{p}_{rr}")
                                nc.scalar.activation(y32[:], ps[:], Relu,
                                                     bias=col(9 + cog), scale=col(7 + cog))
                                rows.append(y32)
                            rm = pB.tile([128, 10, 32], F32, tag="rm4", bufs=2)
                            nc.vector.tensor_tensor(rm[:], rows[0][:], rows[1][:], op=MAX)
                            rmv = rm[:].rearrange("p (w two) b -> p w two b", two=2)
                            pw = pB.tile([128, 5, 32], F32, tag="pw4", bufs=2)
                            nc.vector.tensor_tensor(pw[:], rmv[:, :, 0, :],
                                                    rmv[:, :, 1, :], op=MAX)
                            split4(pB, pw,
                                   p2a[cog][:, 0, p, :, bsl], p2a[cog][:, 1, p, :, bsl],
                                   p2b[cog][:, 0, p, :, bsl], p2b[cog][:, 1, p, :, bsl],
                                   tag="p2s", bufs=2)

            # =============== phase C: L5, L6, FC, softmax ===============
            with tc.tile_pool(name="pC", bufs=1) as pC:
                w6as = [pC.tile([128, 4, 5, 2, 128], F8E4, name=f"w6as{i}") for i in range(4)]
                w6bs = [pC.tile([128, 4, 5, 2, 128], F8E5, name=f"w6bs{i}") for i in range(4)]
                fw2as = pC.tile([128, 8, 4, 2, 128], F8E4)
                fw2bs = pC.tile([128, 8, 4, 2, 128], F8E5)
                fw3as = pC.tile([128, 4, 2, 16], F8E4)
                fw3bs = pC.tile([128, 4, 2, 16], F8E5)
                l5a = [pC.tile([128, 2, 3, 3, B], F8E4, name=f"l5a{i}") for i in range(4)]
                l5b = [pC.tile([128, 2, 3, 3, B], F8E5, name=f"l5b{i}") for i in range(4)]
                fta = pC.tile([128, 2, 4, B], F8E4)
                ftb = pC.tile([128, 2, 4, B], F8E5)
                z1a = pC.tile([128, 2, 8, B], F8E4)
                z1b = pC.tile([128, 2, 8, B], F8E5)
                z2a = pC.tile([128, 2, 8, B], F8E4)
                z2b = pC.tile([128, 2, 8, B], F8E5)
                for i in range(4):
                    nc.sync.dma_start(out=w6as[i][:], in_=w6a[i])
                    nc.sync.dma_start(out=w6bs[i][:], in_=w6b[i])
                nc.sync.dma_start(out=fw2as[:], in_=fw2a[:])
                nc.sync.dma_start(out=fw2bs[:], in_=fw2b[:])
                nc.sync.dma_start(out=fw3as[:], in_=fw3a[:])
                nc.sync.dma_start(out=fw3bs[:], in_=fw3b[:])

                # ---- L5 ----
                for cog in range(4):
                    for h in range(3):
                        ps = psp.tile([128, 3, B], F32, tag="ps")
                        mmo = MM(ps, 36)
                        for cb in range(2):
                            dr9(mmo, w5as[cb][:, cog], p2a[cb], h, 3)
                            dr9(mmo, w5bs[cb][:, cog], p2b[cb], h, 3)
                        y32 = pC.tile([128, 3, B], F32, tag="y5c", bufs=4,
                                      name=f"y5_{cog}_{h}")
                        nc.scalar.activation(y32[:], ps[:], Relu,
                                             bias=col(15 + cog), scale=col(11 + cog))
                        split4(pC, y32,
                               l5a[cog][:, 0, h], l5a[cog][:, 1, h],
                               l5b[cog][:, 0, h], l5b[cog][:, 1, h],
                               tag="l5s")

                # ---- L6 (3x3 conv on 3x3 input == dense over (ci, s)) ----
                for cog in range(4):
                    ps = psp.tile([128, B], F32, tag="ps")
                    mmo = MM(ps, 72)
                    for cb in range(4):
                        for (ws_, part) in ((w6as, l5a), (w6bs, l5b)):
                            pv = part[cb][:].rearrange("p t h w b -> p t (h w) b")
                            for t in range(2):
                                for sp in range(4):
                                    mmo.mm(ws_[cb][:, cog, sp],
                                           pv[:, t, 2 * sp:2 * sp + 2, :], dr=True)
                            mmo.mm(ws_[cb][:, cog, 4], pv[:, 0:2, 8, :], dr=True)
                    y32 = pC.tile([128, B], F32, tag="y6c", bufs=4, name=f"y6_{cog}")
                    nc.scalar.activation(y32[:], ps[:], Relu,
                                         bias=col(23 + cog), scale=col(19 + cog))
                    split4(pC, y32,
                           fta[:, 0, cog], fta[:, 1, cog],
                           ftb[:, 0, cog], ftb[:, 1, cog], tag="fts")

                # ---- FC1 ----
                sc_f1 = _c(SIG[6] / SIG[5])
                for cog in range(8):
                    ps = psp.tile([128, B], F32, tag="ps")
                    mmo = MM(ps, 8)
                    for (ws_, pt) in ((fw1as, fta), (fw1bs, ftb)):
                        for t in range(2):
                            for kp in range(2):
                                mmo.mm(ws_[:, cog, kp],
                                       pt[:, t, 2 * kp:2 * kp + 2, :], dr=True)
                    y32 = pC.tile([128, B], F32, tag="yf1", bufs=4, name=f"yf1_{cog}")
                    nc.scalar.activation(y32[:], ps[:], Relu,
                                         bias=col(27 + cog), scale=sc_f1)
                    split4(pC, y32,
                           z1a[:, 0, cog], z1a[:, 1, cog],
                           z1b[:, 0, cog], z1b[:, 1, cog], tag="z1s")

                # ---- FC2 ----
                sc_f2 = _c(SIG[7] / SIG[6])
                for cog in range(8):
                    ps = psp.tile([128, B], F32, tag="ps")
                    mmo = MM(ps, 16)
                    for (ws_, pt) in ((fw2as, z1a), (fw2bs, z1b)):
                        for t in range(2):
                            for kp in range(4):
                                mmo.mm(ws_[:, cog, kp],
                                       pt[:, t, 2 * kp:2 * kp + 2, :], dr=True)
                    y32 = pC.tile([128, B], F32, tag="yf2", bufs=4, name=f"yf2_{cog}")
                    nc.scalar.activation(y32[:], ps[:], Relu,
                                         bias=col(35 + cog), scale=sc_f2)
                    split4(pC, y32,
                           z2a[:, 0, cog], z2a[:, 1, cog],
                           z2b[:, 0, cog], z2b[:, 1, cog], tag="z2s")

                # ---- FC3 + softmax ----
                ps = psp.tile([16, B], F32, tag="ps")
                mmo = MM(ps, 16)
                for (ws_, pt) in ((fw3as, z2a), (fw3bs, z2b)):
                    for t in range(2):
                        for kp in range(4):
                            mmo.mm(ws_[:, kp], pt[:, t, 2 * kp:2 * kp + 2, :], dr=True)
                logits = pC.tile([10, B], F32)
                nc.vector.tensor_scalar(logits[:], ps[0:10, :], inv_f2, cs[0:10, 43:44],
                                        op0=MULT, op1=ADD)
                pst = psp.tile([B, 10], F32, tag="ps")
                nc.tensor.transpose(pst[:], logits[:], ids[0:10, 0:10])
                nm = pC.tile([B, 1], F32)
                nc.vector.tensor_reduce(out=nm[:], in_=pst[:], op=MAX,
                                        axis=mybir.AxisListType.X, negate=True)
                ex = pC.tile([B, 10], F32)
                sm = pC.tile([B, 1], F32)
                nc.scalar.activation(ex[:], pst[:], Exp, bias=nm[:], scale=1.0,
                                     accum_out=sm[:])
                rc = pC.tile([B, 1], F32)
                nc.vector.reciprocal(rc[:], sm[:])
                so = pC.tile([B, 10], F32)
                nc.vector.tensor_scalar_mul(so[:], ex[:], rc[:])
                nc.sync.dma_start(out=out[:], in_=so[:])

    nc.compile()
    _NC_CACHE["nc"] = nc
    return nc


# ---------------- host-side data prep ----------------

def _fold_bn(b, g, be, m, v):
    inv = (g / np.sqrt(v + EPS)).astype(np.float32)
    return inv, ((b - m) * inv + be).astype(np.float32)


def _w12(w, dt_):
    """[co, ci, 3, 3] +-1 conv weights -> [ci//128, 128, co//128, 6, 2, 128]
    pair-contiguous blocks: k in 0-2 = (dh0,dh1) vertical pair at dw=k;
    k in 3-5 = dh2 tap at dw=k-3, duplicated."""
    co, ci = w.shape[0], w.shape[1]
    ws = np.sign(w).transpose(1, 2, 3, 0)  # [ci, dh, dw, co]
    out = np.zeros((ci // 128, 128, co // 128, 6, 2, 128), dt_)
    for cb in range(ci // 128):
        blk = ws[cb * 128:(cb + 1) * 128]
        for cog in range(co // 128):
            csl = slice(cog * 128, (cog + 1) * 128)
            for dwi in range(3):
                out[cb, :, cog, dwi, 0] = blk[:, 0, dwi, csl].astype(dt_)
                out[cb, :, cog, dwi, 1] = blk[:, 1, dwi, csl].astype(dt_)
                out[cb, :, cog, 3 + dwi, 0] = blk[:, 2, dwi, csl].astype(dt_)
                out[cb, :, cog, 3 + dwi, 1] = blk[:, 2, dwi, csl].astype(dt_)
    return np.ascontiguousarray(out)


def _w10_l6(w, dt_):
    """[512, 512, 3, 3] -> [4, 128, 4, 5, 2, 128]: pair blocks 0-3 =
    (s even, s odd) pairs; block 4 = s8 duplicated."""
    ws = np.sign(w).transpose(1, 2, 3, 0).reshape(512, 9, 512)  # [ci, s, co]
    out = np.zeros((4, 128, 4, 5, 2, 128), dt_)
    for cb in range(4):
        blk = ws[cb * 128:(cb + 1) * 128]
        for cog in range(4):
            csl = slice(cog * 128, (cog + 1) * 128)
            for sp in range(4):
                out[cb, :, cog, sp, 0] = blk[:, 2 * sp, csl].astype(dt_)
                out[cb, :, cog, sp, 1] = blk[:, 2 * sp + 1, csl].astype(dt_)
            out[cb, :, cog, 4, 0] = blk[:, 8, csl].astype(dt_)
            out[cb, :, cog, 4, 1] = blk[:, 8, csl].astype(dt_)
    return np.ascontiguousarray(out)


def _wfc(w, kt, dt_):
    """[co, K] -> [128, co//128 (or 1), kt//2, 2, min(co,128)] pair-contiguous
    k-tile pairs per output block."""
    ws = np.sign(w).T  # [K, co]
    K, co = ws.shape
    kt_ = K // 128
    ncog = max(1, co // 128)
    cw = co // ncog
    out = np.zeros((128, ncog, kt_ // 2, 2, cw), dt_)
    for cog in range(ncog):
        csl = slice(cog * cw, (cog + 1) * cw)
        for kp in range(kt_ // 2):
            for j in range(2):
                kb = 2 * kp + j
                out[:, cog, kp, j] = ws[kb * 128:(kb + 1) * 128, csl].astype(dt_)
    return np.ascontiguousarray(out)


def _prep_shared(inputs):
    d = {}
    # L1: im2col weights with hi/lo x-terms and bias rows (unchanged).
    w1c = np.ascontiguousarray(np.sign(inputs["w1"]).transpose(1, 2, 3, 0).reshape(
        27, 128)).astype(bf16)
    s1f, t1f = _fold_bn(inputs["b1"], inputs["g1"], inputs["be1"],
                        inputs["m1"], inputs["v1"])
    bias_row = (t1f / s1f).astype(np.float32)
    bh = bias_row.astype(bf16)
    bl = (bias_row - bh.astype(np.float32)).astype(bf16)
    d["w1"] = np.vstack([w1c, w1c, bh[None, :], bl[None, :]])

    w2s = np.sign(inputs["w2"]).transpose(1, 2, 3, 0)  # [128, 3, 3, 128]
    d["w2h"] = np.ascontiguousarray(
        w2s.reshape(128, 9, 128)).astype(bf16)
    d["w2e"] = _w12(inputs["w2"], e5m2)[0, :, 0]
    d["w3a"] = _w12(inputs["w3"], e4m3)[0]
    d["w3b"] = _w12(inputs["w3"], e5m2)[0]
    d["w4a"] = _w12(inputs["w4"], e4m3)
    d["w4b"] = _w12(inputs["w4"], e5m2)
    d["w5a"] = _w12(inputs["w5"], e4m3)
    d["w5b"] = _w12(inputs["w5"], e5m2)
    d["w6a"] = _w10_l6(inputs["w6"], e4m3)
    d["w6b"] = _w10_l6(inputs["w6"], e5m2)
    d["fw1a"] = _wfc(inputs["fw1"], 4, e4m3)
    d["fw1b"] = _wfc(inputs["fw1"], 4, e5m2)
    d["fw2a"] = _wfc(inputs["fw2"], 8, e4m3)
    d["fw2b"] = _wfc(inputs["fw2"], 8, e5m2)
    fw3p = np.zeros((16, 1024), np.float32)
    fw3p[:10] = np.sign(inputs["fw3"])
    d["fw3a"] = _wfc(fw3p, 8, e4m3)[:, 0]
    d["fw3b"] = _wfc(fw3p, 8, e5m2)[:, 0]

    consts = np.zeros((128, 44), np.float32)
    consts[:, 0] = s1f * SIG[0]
    # conv layers 2..6: scale = s*sig/sig_prev, bias = t*sig
    coff = [(2, 1, 2, 1), (3, 3, 5, 2), (4, 7, 9, 3), (5, 11, 15, 4),
            (6, 19, 23, 5)]
    for li, so, to, sigi in coff:
        s, t = _fold_bn(inputs[f"b{li}"], inputs[f"g{li}"], inputs[f"be{li}"],
                        inputs[f"m{li}"], inputs[f"v{li}"])
        sc = (s * (SIG[sigi] / SIG[sigi - 1])).astype(np.float32)
        tb = (t * SIG[sigi]).astype(np.float32)
        nb = len(s) // 128
        for j in range(nb):
            consts[:, so + j] = sc[j * 128:(j + 1) * 128]
            consts[:, to + j] = tb[j * 128:(j + 1) * 128]
    for j in range(8):
        consts[:, 27 + j] = inputs["fb1"][j * 128:(j + 1) * 128] * SIG[6]
        consts[:, 35 + j] = inputs["fb2"][j * 128:(j + 1) * 128] * SIG[7]
    consts[0:10, 43] = inputs["fb3"]
    d["consts"] = consts
    d["ident"] = np.eye(16, dtype=np.float32)
    return d


def _prep_x(xc):
    # xc [B, 3, 32, 32] f32 -> im2col [N_SB, 56, 30*30*SB] bf16 (hi 0-26, lo 27-53)
    x32 = xc.astype(np.float32)
    hi = x32.astype(bf16)
    lo = (x32 - hi.astype(np.float32)).astype(bf16)
    parts = []
    for p in (hi, lo):
        win = np.lib.stride_tricks.sliding_window_view(p, (3, 3), axis=(2, 3))
        arr = win.transpose(1, 4, 5, 2, 3, 0).reshape(27, 30, 30, B)
        parts.append(arr)
    ones = np.ones((2, 30, 30, B), bf16)
    full = np.concatenate(parts + [ones], axis=0)  # [56, 30, 30, B]
    full = full.reshape(56, 30, 30, N_SB, SB).transpose(3, 0, 1, 2, 4)
    return np.ascontiguousarray(full).reshape(N_SB, 56, 30 * 30 * SB)


def make_in_maps(inputs):
    shared = _prep_shared(inputs)
    x = np.asarray(inputs["x"])
    in_maps = []
    for c in range(N_CORES):
        m = dict(shared)
        m["xi"] = _prep_x(x[c * B:(c + 1) * B])
        in_maps.append(m)
    return in_maps


def kernel(**inputs):
    nc = build_nc()
    in_maps = make_in_maps(inputs)
    res = run_bass_kernel_spmd(nc, in_maps, list(range(N_CORES)))
    return np.concatenate([res.results[c]["out"] for c in range(N_CORES)], axis=0)


# revision 13
# speedup vs baseline: 1.1747x; 1.0016x over previous
# Trainium2 Bass kernel for nn_BinaryConv (binarized VGG-ish CNN, batch 512).
#
# Strategy: pure data parallel over 8 NeuronCores (64 images each), weights
# replicated. Precision: every layer's activations are decomposed into terms
# whose matmuls accumulate exactly in fp32 PSUM:
#   - L2 input (L1 output): bf16 hi term `h` + e5m2 residual pair (t2,t3)
#     -> 9 bf16 matmuls + 9 fp8 DoubleRow matmuls per output row chunk.
#   - L3..L6 / FC inputs: 4-term fp8 split: e4m3 pair (t0,t1) + e5m2 pair
#     (t2,t3), consumed exclusively with fp8 DoubleRow matmuls (2 k-tiles
#     per instruction at 0.5 cycles/row = 4x bf16 throughput per k-tile).
# Effective mantissa ~16 bits, matching the fp32 reference closely enough
# to reproduce its (exactly one-hot) softmax output.
#
# DoubleRow pairing: vertical tap pairs (dh=0,1 at fixed dw) ride adjacent
# h-slices of the stored activation tile; the leftover dh=2 taps pair the
# two split terms of one tap (weights duplicated host-side for those rows).
# Per-layer power-of-2 output scales keep e4m3 terms below its 240 max.
# BN+bias are folded into per-channel scale/bias applied out of PSUM.

import numpy as np
import ml_dtypes

import concourse.mybir as mybir
import concourse.tile as tile
from concourse import bacc
from concourse.bass_utils import run_bass_kernel_spmd

bf16 = ml_dtypes.bfloat16
e4m3 = ml_dtypes.float8_e4m3
e5m2 = ml_dtypes.float8_e5m2
F32 = mybir.dt.float32
BF16 = mybir.dt.bfloat16
F8E4 = mybir.dt.float8e4
F8E5 = mybir.dt.float8e5
Relu = mybir.ActivationFunctionType.Relu
Copy = mybir.ActivationFunctionType.Copy
Exp = mybir.ActivationFunctionType.Exp
MULT = mybir.AluOpType.mult
SUB = mybir.AluOpType.subtract
MAX = mybir.AluOpType.max
ADD = mybir.AluOpType.add
DR = mybir.MatmulPerfMode.DoubleRow

N_CORES = 8
B = 64          # images per core
SB = 16         # L1/L2 sub-batch
N_SB = 4
EPS = 1e-5

# Per-layer power-of-2 output scales (stored activation = SIG[l] * true).
# Chosen so each scaled tensor's max stays well under e4m3's 240 limit.
SIG = [2.0 ** e for e in (1, -4, -9, -14, -19, -25, -28, -33)]
# y1, p1, l3, l4(p2), l5, l6, fc1, fc2 output scales (validated in proto)

_NC_CACHE = {}


def _c(v):
    return float(np.float32(v))


def build_nc():
    if "nc" in _NC_CACHE:
        return _NC_CACHE["nc"]
    nc = bacc.Bacc(None, target_bir_lowering=False, debug=False)

    # ---------------- DRAM parameters ----------------
    xi = nc.declare_dram_parameter("xi", [N_SB, 83, 30 * 30 * SB], BF16, isOutput=False)
    w1 = nc.declare_dram_parameter("w1", [83, 128], BF16, isOutput=False)
    w2h = nc.declare_dram_parameter("w2h", [128, 9, 128], BF16, isOutput=False)
    w2e = nc.declare_dram_parameter("w2e", [128, 6, 2, 128], F8E5, isOutput=False)
    w3a = nc.declare_dram_parameter("w3a", [128, 2, 6, 2, 128], F8E4, isOutput=False)
    w3b = nc.declare_dram_parameter("w3b", [128, 2, 6, 2, 128], F8E5, isOutput=False)
    w4a = nc.declare_dram_parameter("w4a", [2, 128, 2, 6, 2, 128], F8E4, isOutput=False)
    w4b = nc.declare_dram_parameter("w4b", [2, 128, 2, 6, 2, 128], F8E5, isOutput=False)
    w5a = nc.declare_dram_parameter("w5a", [2, 128, 4, 6, 2, 128], F8E4, isOutput=False)
    w5b = nc.declare_dram_parameter("w5b", [2, 128, 4, 6, 2, 128], F8E5, isOutput=False)
    w6a = nc.declare_dram_parameter("w6a", [4, 128, 4, 5, 2, 128], F8E4, isOutput=False)
    w6b = nc.declare_dram_parameter("w6b", [4, 128, 4, 5, 2, 128], F8E5, isOutput=False)
    fw1a = nc.declare_dram_parameter("fw1a", [128, 8, 2, 2, 128], F8E4, isOutput=False)
    fw1b = nc.declare_dram_parameter("fw1b", [128, 8, 2, 2, 128], F8E5, isOutput=False)
    fw2a = nc.declare_dram_parameter("fw2a", [128, 8, 4, 2, 128], F8E4, isOutput=False)
    fw2b = nc.declare_dram_parameter("fw2b", [128, 8, 4, 2, 128], F8E5, isOutput=False)
    fw3a = nc.declare_dram_parameter("fw3a", [128, 4, 2, 16], F8E4, isOutput=False)
    fw3b = nc.declare_dram_parameter("fw3b", [128, 4, 2, 16], F8E5, isOutput=False)
    # consts columns: 0:s1 1:s2 2:t2 3-4:s3 5-6:t3 7-8:s4 9-10:t4
    # 11-14:s5 15-18:t5 19-22:s6 23-26:t6 27-34:fb1 35-42:fb2 43:fb3(rows0-9)
    consts = nc.declare_dram_parameter("consts", [128, 44], F32, isOutput=False)
    ident = nc.declare_dram_parameter("ident", [16, 16], F32, isOutput=False)
    out = nc.declare_dram_parameter("out", [B, 10], F32, isOutput=True)

    inv_f2 = _c(1.0 / SIG[7])

    with tile.TileContext(nc) as tc:
        with tc.tile_pool(name="psp", bufs=8, space="PSUM") as psp, \
             tc.tile_pool(name="p0", bufs=1) as p0:
            # ---------------- persistent tiles ----------------
            w1s = p0.tile([83, 128], BF16)
            w2hs = p0.tile([128, 9, 128], BF16)
            w2es = p0.tile([128, 6, 2, 128], F8E5)
            w3as = p0.tile([128, 2, 6, 2, 128], F8E4)
            w3bs = p0.tile([128, 2, 6, 2, 128], F8E5)
            w4as = [p0.tile([128, 2, 6, 2, 128], F8E4, name=f"w4as{i}") for i in range(2)]
            w4bs = [p0.tile([128, 2, 6, 2, 128], F8E5, name=f"w4bs{i}") for i in range(2)]
            w5as = [p0.tile([128, 4, 6, 2, 128], F8E4, name=f"w5as{i}") for i in range(2)]
            w5bs = [p0.tile([128, 4, 6, 2, 128], F8E5, name=f"w5bs{i}") for i in range(2)]
            fw1as = p0.tile([128, 8, 2, 2, 128], F8E4)
            fw1bs = p0.tile([128, 8, 2, 2, 128], F8E5)
            cs = p0.tile([128, 44], F32)
            ids = p0.tile([16, 16], F32)
            p1a = [p0.tile([128, 2, 14, 14, 32], F8E4, name=f"p1a{i}") for i in range(2)]
            p1b = [p0.tile([128, 2, 14, 14, 32], F8E5, name=f"p1b{i}") for i in range(2)]
            p2a = [p0.tile([128, 2, 5, 5, B], F8E4, name=f"p2a{i}") for i in range(2)]
            p2b = [p0.tile([128, 2, 5, 5, B], F8E5, name=f"p2b{i}") for i in range(2)]

            nc.sync.dma_start(out=w1s[:], in_=w1[:])
            nc.sync.dma_start(out=w2hs[:], in_=w2h[:])
            nc.sync.dma_start(out=w2es[:], in_=w2e[:])
            nc.sync.dma_start(out=cs[:], in_=consts[:])
            nc.sync.dma_start(out=ids[:], in_=ident[:])

            def load_phaseb_weights():
                nc.sync.dma_start(out=w3as[:], in_=w3a[:])
                nc.sync.dma_start(out=w3bs[:], in_=w3b[:])
                for i in range(2):
                    nc.sync.dma_start(out=w4as[i][:], in_=w4a[i])
                    nc.sync.dma_start(out=w4bs[i][:], in_=w4b[i])

            def load_phasec_weights():
                for i in range(2):
                    nc.sync.dma_start(out=w5as[i][:], in_=w5a[i])
                    nc.sync.dma_start(out=w5bs[i][:], in_=w5b[i])
                nc.sync.dma_start(out=fw1as[:], in_=fw1a[:])
                nc.sync.dma_start(out=fw1bs[:], in_=fw1b[:])

            def col(j):
                return cs[:, j:j + 1]

            class MM:
                """start/stop bookkeeping for one PSUM accumulation group."""
                def __init__(self, ps, n):
                    self.ps, self.n, self.i = ps, n, 0

                def mm(self, lhsT, rhs, dr=False):
                    nc.tensor.matmul(self.ps[:], lhsT, rhs,
                                     start=(self.i == 0), stop=(self.i == self.n - 1),
                                     perf_mode=DR if dr else None)
                    self.i += 1

            def dr9(mmo, wa_, pt, hsl, wlen):
                """9 DoubleRow matmuls covering 2 terms x 9 taps of one dtype.

                pt: [128, 2, H, W, Bd] tile; hsl: first input row; window w
                length wlen. Weights wa_: [128, 6, 2, M] pair-contiguous
                (blocks 0-2: vertical dh01 pairs per dwi; 3-5: dh2 dups)."""
                for t in range(2):
                    for dwi in range(3):
                        mmo.mm(wa_[:, dwi],
                               pt[:, t, hsl:hsl + 2, dwi:dwi + wlen, :], dr=True)
                for dwi in range(3):
                    mmo.mm(wa_[:, 3 + dwi],
                           pt[:, 0:2, hsl + 2, dwi:dwi + wlen, :], dr=True)

            def split4(pool, y32, ta0, ta1, tb0, tb1, tag, bufs=3):
                """4-term split of relu'd scaled fp32 y32 into e4m3 pair
                (ta0, ta1) + e5m2 pair (tb0, tb1).
                Engines: pure casts ride gpsimd software-DGE casting DMAs
                (Pool engine posts descriptors, transfer on idle DMA engines);
                subtracts on DVE; one cast each on ACT (and optionally t0)."""
                shp = [y32.tensor.shape[0]] + list(y32.tensor.shape[1:])
                nc.scalar.activation(ta0, y32[:], Relu)
                r1 = pool.tile(shp, F32, tag=f"{tag}_r1", bufs=bufs)
                nc.vector.scalar_tensor_tensor(r1[:], y32[:], 1.0, ta0,
                                               op0=MULT, op1=SUB)
                nc.gpsimd.dma_start(out=ta1, in_=r1[:])
                r2 = pool.tile(shp, F32, tag=f"{tag}_r2", bufs=bufs)
                nc.vector.scalar_tensor_tensor(r2[:], r1[:], 1.0, ta1,
                                               op0=MULT, op1=SUB)
                nc.scalar.activation(tb0, r2[:], Copy)
                nc.vector.scalar_tensor_tensor(tb1, r2[:], 1.0, tb0,
                                               op0=MULT, op1=SUB)

            # =============== phase A: L1, L2, pool1 (per sub-batch) ===============
            with tc.tile_pool(name="pA", bufs=1) as pA:
                l1h = pA.tile([128, 30, 30, SB], BF16, tag="l1h")
                l1e = pA.tile([128, 2, 30, 30, SB], F8E5, tag="l1e")
                prev_row = [None]

                def l1_row(sb, r):
                    ic = pA.tile([83, 30, SB], BF16, tag="ic", bufs=5, name=f"ic_{sb}_{r}")
                    nc.sync.dma_start(
                        out=ic[:], in_=xi[sb, :, r * 30 * SB:(r + 1) * 30 * SB])
                    ps = psp.tile([128, 30, SB], F32, tag="ps", name=f"ps1_{sb}_{r}")
                    nc.tensor.matmul(ps[:], w1s[:], ic[:], start=True, stop=True)
                    # s1*sig1 is folded into w1 host-side: ps is scaled+biased.
                    nc.scalar.activation(l1h[:, r], ps[:], Relu)
                    l32 = pA.tile([128, 30, SB], F32, tag="l32", bufs=3, name=f"lw_{sb}_{r}")
                    nc.vector.scalar_tensor_tensor(l32[:], ps[:], 0.0, l1h[:, r],
                                                   op0=MAX, op1=SUB)
                    nc.scalar.activation(l1e[:, 0, r], l32[:], Copy)
                    nc.vector.scalar_tensor_tensor(l1e[:, 1, r], l32[:], 1.0,
                                                   l1e[:, 0, r], op0=MULT, op1=SUB)

                def l2_row(sb, q):
                    bsl = slice(sb * SB, (sb + 1) * SB)
                    ps = psp.tile([128, 28, SB], F32, tag="ps", name=f"ps2_{sb}_{q}")
                    mmo = MM(ps, 18)
                    for dh in range(3):
                        for dw in range(3):
                            mmo.mm(w2hs[:, dh * 3 + dw, :],
                                   l1h[:, q + dh, dw:dw + 28, :])
                    dr9(mmo, w2es, l1e, q, 28)
                    y32 = pA.tile([128, 28, SB], F32, tag="y2", bufs=4,
                                  name=f"y2_{sb}_{q}")
                    nc.scalar.activation(y32[:], ps[:], Relu, bias=col(2), scale=col(1))
                    if q % 2 == 0:
                        prev_row[0] = y32
                        return
                    p = q // 2
                    rm = pA.tile([128, 28, SB], F32, tag="rm", bufs=2, name=f"rm_{sb}_{p}")
                    nc.vector.tensor_tensor(rm[:], prev_row[0][:], y32[:], op=MAX)
                    rmv = rm[:].rearrange("p (w two) b -> p w two b", two=2)
                    pw = pA.tile([128, 14, SB], F32, tag="pw", bufs=2, name=f"pw_{sb}_{p}")
                    nc.vector.tensor_tensor(pw[:], rmv[:, :, 0, :], rmv[:, :, 1, :], op=MAX)
                    hb, hsl = sb // 2, slice((sb % 2) * SB, (sb % 2 + 1) * SB)
                    split4(pA, pw,
                           p1a[hb][:, 0, p, :, hsl], p1a[hb][:, 1, p, :, hsl],
                           p1b[hb][:, 0, p, :, hsl], p1b[hb][:, 1, p, :, hsl],
                           tag="p1s", bufs=2)

                SKEW = 4
                for gi in range(N_SB * 30 + SKEW):
                    if gi < N_SB * 30:
                        sb1, r = divmod(gi, 30)
                        l1_row(sb1, r)
                    if gi == 12:
                        load_phaseb_weights()
                    gq = gi - SKEW
                    if gq >= 0:
                        sb2, q = divmod(gq, 30)
                        if q < 28:
                            l2_row(sb2, q)

            # ====== phase B: L3, L4, pool2 (split over batch halves) ======
            with tc.tile_pool(name="pB", bufs=1) as pB:
                for bh in range(2):
                    bsl = slice(bh * 32, (bh + 1) * 32)
                    l3a = [pB.tile([128, 2, 12, 12, 32], F8E4, tag=f"l3a{i}",
                                   name=f"l3a{i}_{bh}") for i in range(2)]
                    l3b = [pB.tile([128, 2, 12, 12, 32], F8E5, tag=f"l3b{i}",
                                   name=f"l3b{i}_{bh}") for i in range(2)]
                    # ---- L3 ----
                    for cog in range(2):
                        for r in range(12):
                            ps = psp.tile([128, 12, 32], F32, tag="ps")
                            mmo = MM(ps, 18)
                            dr9(mmo, w3as[:, cog], p1a[bh], r, 12)
                            dr9(mmo, w3bs[:, cog], p1b[bh], r, 12)
                            y32 = pB.tile([128, 12, 32], F32, tag="y3b", bufs=3,
                                          name=f"y3_{cog}_{r}_{bh}")
                            nc.scalar.activation(y32[:], ps[:], Relu,
                                                 bias=col(5 + cog), scale=col(3 + cog))
                            split4(pB, y32,
                                   l3a[cog][:, 0, r], l3a[cog][:, 1, r],
                                   l3b[cog][:, 0, r], l3b[cog][:, 1, r],
                                   tag="l3s", bufs=2)
                            if bh == 0 and cog == 1 and r == 5:
                                load_phasec_weights()
                    # ---- L4 + pool2 ----
                    for cog in range(2):
                        for p in range(5):
                            rows = []
                            for rr in range(2):
                                r = 2 * p + rr
                                ps = psp.tile([128, 10, 32], F32, tag="ps")
                                mmo = MM(ps, 36)
                                for cb in range(2):
                                    dr9(mmo, w4as[cb][:, cog], l3a[cb], r, 10)
                                    dr9(mmo, w4bs[cb][:, cog], l3b[cb], r, 10)
                                y32 = pB.tile([128, 10, 32], F32, tag="y4b", bufs=3,
                                              name=f"y4_{cog}_{bh}_{p}_{rr}")
                                nc.scalar.activation(y32[:], ps[:], Relu,
                                                     bias=col(9 + cog), scale=col(7 + cog))
                                rows.append(y32)
                            rm = pB.tile([128, 10, 32], F32, tag="rm4", bufs=2)
                            nc.vector.tensor_tensor(rm[:], rows[0][:], rows[1][:], op=MAX)
                            rmv = rm[:].rearrange("p (w two) b -> p w two b", two=2)
                            pw = pB.tile([128, 5, 32], F32, tag="pw4", bufs=2)
                            nc.vector.tensor_tensor(pw[:], rmv[:, :, 0, :],
                                                    rmv[:, :, 1, :], op=MAX)
                            split4(pB, pw,
                                   p2a[cog][:, 0, p, :, bsl], p2a[cog][:, 1, p, :, bsl],
                                   p2b[cog][:, 0, p, :, bsl], p2b[cog][:, 1, p, :, bsl],
                                   tag="p2s", bufs=2)

            # =============== phase C: L5, L6, FC, softmax ===============
            with tc.tile_pool(name="pC", bufs=1) as pC:
                w6as = [pC.tile([128, 4, 5, 2, 128], F8E4, name=f"w6as{i}") for i in range(4)]
                w6bs = [pC.tile([128, 4, 5, 2, 128], F8E5, name=f"w6bs{i}") for i in range(4)]
                fw2as = pC.tile([128, 8, 4, 2, 128], F8E4)
                fw2bs = pC.tile([128, 8, 4, 2, 128], F8E5)
                fw3as = pC.tile([128, 4, 2, 16], F8E4)
                fw3bs = pC.tile([128, 4, 2, 16], F8E5)
                l5a = [pC.tile([128, 2, 3, 3, B], F8E4, name=f"l5a{i}") for i in range(4)]
                l5b = [pC.tile([128, 2, 3, 3, B], F8E5, name=f"l5b{i}") for i in range(4)]
                fta = pC.tile([128, 2, 4, B], F8E4)
                ftb = pC.tile([128, 2, 4, B], F8E5)
                z1a = pC.tile([128, 2, 8, B], F8E4)
                z1b = pC.tile([128, 2, 8, B], F8E5)
                z2a = pC.tile([128, 2, 8, B], F8E4)
                z2b = pC.tile([128, 2, 8, B], F8E5)
                for i in range(4):
                    nc.sync.dma_start(out=w6as[i][:], in_=w6a[i])
                    nc.sync.dma_start(out=w6bs[i][:], in_=w6b[i])
                nc.sync.dma_start(out=fw2as[:], in_=fw2a[:])
                nc.sync.dma_start(out=fw2bs[:], in_=fw2b[:])
                nc.sync.dma_start(out=fw3as[:], in_=fw3a[:])
                nc.sync.dma_start(out=fw3bs[:], in_=fw3b[:])

                # ---- L5 ----
                for cog in range(4):
                    for h in range(3):
                        ps = psp.tile([128, 3, B], F32, tag="ps")
                        mmo = MM(ps, 36)
                        for cb in range(2):
                            dr9(mmo, w5as[cb][:, cog], p2a[cb], h, 3)
                            dr9(mmo, w5bs[cb][:, cog], p2b[cb], h, 3)
                        y32 = pC.tile([128, 3, B], F32, tag="y5c", bufs=4,
                                      name=f"y5_{cog}_{h}")
                        nc.scalar.activation(y32[:], ps[:], Relu,
                                             bias=col(15 + cog), scale=col(11 + cog))
                        split4(pC, y32,
                               l5a[cog][:, 0, h], l5a[cog][:, 1, h],
                               l5b[cog][:, 0, h], l5b[cog][:, 1, h],
                               tag="l5s")

                # ---- L6 (3x3 conv on 3x3 input == dense over (ci, s)) ----
                for cog in range(4):
                    ps = psp.tile([128, B], F32, tag="ps")
                    mmo = MM(ps, 72)
                    for cb in range(4):
                        for (ws_, part) in ((w6as, l5a), (w6bs, l5b)):
                            pv = part[cb][:].rearrange("p t h w b -> p t (h w) b")
                            for t in range(2):
                                for sp in range(4):
                                    mmo.mm(ws_[cb][:, cog, sp],
                                           pv[:, t, 2 * sp:2 * sp + 2, :], dr=True)
                            mmo.mm(ws_[cb][:, cog, 4], pv[:, 0:2, 8, :], dr=True)
                    y32 = pC.tile([128, B], F32, tag="y6c", bufs=4, name=f"y6_{cog}")
                    nc.scalar.activation(y32[:], ps[:], Relu,
                                         bias=col(23 + cog), scale=col(19 + cog))
                    split4(pC, y32,
                           fta[:, 0, cog], fta[:, 1, cog],
                           ftb[:, 0, cog], ftb[:, 1, cog], tag="fts")

                # ---- FC1 ----
                sc_f1 = _c(SIG[6] / SIG[5])
                for cog in range(8):
                    ps = psp.tile([128, B], F32, tag="ps")
                    mmo = MM(ps, 8)
                    for (ws_, pt) in ((fw1as, fta), (fw1bs, ftb)):
                        for t in range(2):
                            for kp in range(2):
                                mmo.mm(ws_[:, cog, kp],
                                       pt[:, t, 2 * kp:2 * kp + 2, :], dr=True)
                    y32 = pC.tile([128, B], F32, tag="yf1", bufs=4, name=f"yf1_{cog}")
                    nc.scalar.activation(y32[:], ps[:], Relu,
                                         bias=col(27 + cog), scale=sc_f1)
                    split4(pC, y32,
                           z1a[:, 0, cog], z1a[:, 1, cog],
                           z1b[:, 0, cog], z1b[:, 1, cog], tag="z1s")

                # ---- FC2 ----
                sc_f2 = _c(SIG[7] / SIG[6])
                for cog in range(8):
                    ps = psp.tile([128, B], F32, tag="ps")
                    mmo = MM(ps, 16)
                    for (ws_, pt) in ((fw2as, z1a), (fw2bs, z1b)):
                        for t in range(2):
                            for kp in range(4):
                                mmo.mm(ws_[:, cog, kp],
                                       pt[:, t, 2 * kp:2 * kp + 2, :], dr=True)
                    y32 = pC.tile([128, B], F32, tag="yf2", bufs=4, name=f"yf2_{cog}")
                    nc.scalar.activation(y32[:], ps[:], Relu,
                                         bias=col(35 + cog), scale=sc_f2)
                    split4(pC, y32,
                           z2a[:, 0, cog], z2a[:, 1, cog],
                           z2b[:, 0, cog], z2b[:, 1, cog], tag="z2s")

                # ---- FC3 + softmax ----
                ps = psp.tile([16, B], F32, tag="ps")
                mmo = MM(ps, 16)
                for (ws_, pt) in ((fw3as, z2a), (fw3bs, z2b)):
                    for t in range(2):
                        for kp in range(4):
                            mmo.mm(ws_[:, kp], pt[:, t, 2 * kp:2 * kp + 2, :], dr=True)
                logits = pC.tile([10, B], F32)
                nc.vector.tensor_scalar(logits[:], ps[0:10, :], inv_f2, cs[0:10, 43:44],
                                        op0=MULT, op1=ADD)
                pst = psp.tile([B, 10], F32, tag="ps")
                nc.tensor.transpose(pst[:], logits[:], ids[0:10, 0:10])
                nm = pC.tile([B, 1], F32)
                nc.vector.tensor_reduce(out=nm[:], in_=pst[:], op=MAX,
                                        axis=mybir.AxisListType.X, negate=True)
                ex = pC.tile([B, 10], F32)
                sm = pC.tile([B, 1], F32)
                nc.scalar.activation(ex[:], pst[:], Exp, bias=nm[:], scale=1.0,
                                     accum_out=sm[:])
                rc = pC.tile([B, 1], F32)
                nc.vector.reciprocal(rc[:], sm[:])
                so = pC.tile([B, 10], F32)
                nc.vector.tensor_scalar_mul(so[:], ex[:], rc[:])
                nc.sync.dma_start(out=out[:], in_=so[:])

    nc.compile()
    _NC_CACHE["nc"] = nc
    return nc


# ---------------- host-side data prep ----------------

def _fold_bn(b, g, be, m, v):
    inv = (g / np.sqrt(v + EPS)).astype(np.float32)
    return inv, ((b - m) * inv + be).astype(np.float32)


def _w12(w, dt_):
    """[co, ci, 3, 3] +-1 conv weights -> [ci//128, 128, co//128, 6, 2, 128]
    pair-contiguous blocks: k in 0-2 = (dh0,dh1) vertical pair at dw=k;
    k in 3-5 = dh2 tap at dw=k-3, duplicated."""
    co, ci = w.shape[0], w.shape[1]
    ws = np.sign(w).transpose(1, 2, 3, 0)  # [ci, dh, dw, co]
    out = np.zeros((ci // 128, 128, co // 128, 6, 2, 128), dt_)
    for cb in range(ci // 128):
        blk = ws[cb * 128:(cb + 1) * 128]
        for cog in range(co // 128):
            csl = slice(cog * 128, (cog + 1) * 128)
            for dwi in range(3):
                out[cb, :, cog, dwi, 0] = blk[:, 0, dwi, csl].astype(dt_)
                out[cb, :, cog, dwi, 1] = blk[:, 1, dwi, csl].astype(dt_)
                out[cb, :, cog, 3 + dwi, 0] = blk[:, 2, dwi, csl].astype(dt_)
                out[cb, :, cog, 3 + dwi, 1] = blk[:, 2, dwi, csl].astype(dt_)
    return np.ascontiguousarray(out)


def _w10_l6(w, dt_):
    """[512, 512, 3, 3] -> [4, 128, 4, 5, 2, 128]: pair blocks 0-3 =
    (s even, s odd) pairs; block 4 = s8 duplicated."""
    ws = np.sign(w).transpose(1, 2, 3, 0).reshape(512, 9, 512)  # [ci, s, co]
    out = np.zeros((4, 128, 4, 5, 2, 128), dt_)
    for cb in range(4):
        blk = ws[cb * 128:(cb + 1) * 128]
        for cog in range(4):
            csl = slice(cog * 128, (cog + 1) * 128)
            for sp in range(4):
                out[cb, :, cog, sp, 0] = blk[:, 2 * sp, csl].astype(dt_)
                out[cb, :, cog, sp, 1] = blk[:, 2 * sp + 1, csl].astype(dt_)
            out[cb, :, cog, 4, 0] = blk[:, 8, csl].astype(dt_)
            out[cb, :, cog, 4, 1] = blk[:, 8, csl].astype(dt_)
    return np.ascontiguousarray(out)


def _wfc(w, kt, dt_):
    """[co, K] -> [128, co//128 (or 1), kt//2, 2, min(co,128)] pair-contiguous
    k-tile pairs per output block."""
    ws = np.sign(w).T  # [K, co]
    K, co = ws.shape
    kt_ = K // 128
    ncog = max(1, co // 128)
    cw = co // ncog
    out = np.zeros((128, ncog, kt_ // 2, 2, cw), dt_)
    for cog in range(ncog):
        csl = slice(cog * cw, (cog + 1) * cw)
        for kp in range(kt_ // 2):
            for j in range(2):
                kb = 2 * kp + j
                out[:, cog, kp, j] = ws[kb * 128:(kb + 1) * 128, csl].astype(dt_)
    return np.ascontiguousarray(out)


def _prep_shared(inputs):
    d = {}
    # L1: s1*sig1 folded into the weights (bf16 hi/lo rows); bias rows carry
    # t1*sig1. Rows: 27 whi (pair with x-hi), 27 whi (x-lo), 27 wlo (x-hi),
    # 2 bias. The dropped wlo*xlo term is ~2^-18 relative.
    s1f, t1f = _fold_bn(inputs["b1"], inputs["g1"], inputs["be1"],
                        inputs["m1"], inputs["v1"])
    w1c = np.sign(inputs["w1"]).transpose(1, 2, 3, 0).reshape(27, 128)
    w1sc = (w1c * (s1f * SIG[0])[None, :]).astype(np.float32)
    whi = w1sc.astype(bf16)
    wlo = (w1sc - whi.astype(np.float32)).astype(bf16)
    bias_row = (t1f * SIG[0]).astype(np.float32)
    bh = bias_row.astype(bf16)
    bl = (bias_row - bh.astype(np.float32)).astype(bf16)
    d["w1"] = np.vstack([whi, whi, wlo, bh[None, :], bl[None, :]])

    w2s = np.sign(inputs["w2"]).transpose(1, 2, 3, 0)  # [128, 3, 3, 128]
    d["w2h"] = np.ascontiguousarray(
        w2s.reshape(128, 9, 128)).astype(bf16)
    d["w2e"] = _w12(inputs["w2"], e5m2)[0, :, 0]
    d["w3a"] = _w12(inputs["w3"], e4m3)[0]
    d["w3b"] = _w12(inputs["w3"], e5m2)[0]
    d["w4a"] = _w12(inputs["w4"], e4m3)
    d["w4b"] = _w12(inputs["w4"], e5m2)
    d["w5a"] = _w12(inputs["w5"], e4m3)
    d["w5b"] = _w12(inputs["w5"], e5m2)
    d["w6a"] = _w10_l6(inputs["w6"], e4m3)
    d["w6b"] = _w10_l6(inputs["w6"], e5m2)
    d["fw1a"] = _wfc(inputs["fw1"], 4, e4m3)
    d["fw1b"] = _wfc(inputs["fw1"], 4, e5m2)
    d["fw2a"] = _wfc(inputs["fw2"], 8, e4m3)
    d["fw2b"] = _wfc(inputs["fw2"], 8, e5m2)
    fw3p = np.zeros((16, 1024), np.float32)
    fw3p[:10] = np.sign(inputs["fw3"])
    d["fw3a"] = _wfc(fw3p, 8, e4m3)[:, 0]
    d["fw3b"] = _wfc(fw3p, 8, e5m2)[:, 0]

    consts = np.zeros((128, 44), np.float32)
    # conv layers 2..6: scale = s*sig/sig_prev, bias = t*sig
    coff = [(2, 1, 2, 1), (3, 3, 5, 2), (4, 7, 9, 3), (5, 11, 15, 4),
            (6, 19, 23, 5)]
    for li, so, to, sigi in coff:
        s, t = _fold_bn(inputs[f"b{li}"], inputs[f"g{li}"], inputs[f"be{li}"],
                        inputs[f"m{li}"], inputs[f"v{li}"])
        sc = (s * (SIG[sigi] / SIG[sigi - 1])).astype(np.float32)
        tb = (t * SIG[sigi]).astype(np.float32)
        nb = len(s) // 128
        for j in range(nb):
            consts[:, so + j] = sc[j * 128:(j + 1) * 128]
            consts[:, to + j] = tb[j * 128:(j + 1) * 128]
    for j in range(8):
        consts[:, 27 + j] = inputs["fb1"][j * 128:(j + 1) * 128] * SIG[6]
        consts[:, 35 + j] = inputs["fb2"][j * 128:(j + 1) * 128] * SIG[7]
    consts[0:10, 43] = inputs["fb3"]
    d["consts"] = consts
    d["ident"] = np.eye(16, dtype=np.float32)
    return d


def _prep_x(xc):
    # xc [B, 3, 32, 32] f32 -> im2col [N_SB, 83, 30*30*SB] bf16:
    # rows 0-26 x-hi (vs whi), 27-53 x-lo (vs whi), 54-80 x-hi (vs wlo),
    # 81-82 ones (bias rows).
    x32 = xc.astype(np.float32)
    hi = x32.astype(bf16)
    lo = (x32 - hi.astype(np.float32)).astype(bf16)
    parts = []
    for p in (hi, lo, hi):
        win = np.lib.stride_tricks.sliding_window_view(p, (3, 3), axis=(2, 3))
        arr = win.transpose(1, 4, 5, 2, 3, 0).reshape(27, 30, 30, B)
        parts.append(arr)
    ones = np.ones((2, 30, 30, B), bf16)
    full = np.concatenate(parts + [ones], axis=0)  # [83, 30, 30, B]
    full = full.reshape(83, 30, 30, N_SB, SB).transpose(3, 0, 1, 2, 4)
    return np.ascontiguousarray(full).reshape(N_SB, 83, 30 * 30 * SB)


def make_in_maps(inputs):
    shared = _prep_shared(inputs)
    x = np.asarray(inputs["x"])
    in_maps = []
    for c in range(N_CORES):
        m = dict(shared)
        m["xi"] = _prep_x(x[c * B:(c + 1) * B])
        in_maps.append(m)
    return in_maps


def kernel(**inputs):
    nc = build_nc()
    in_maps = make_in_maps(inputs)
    res = run_bass_kernel_spmd(nc, in_maps, list(range(N_CORES)))
    return np.concatenate([res.results[c]["out"] for c in range(N_CORES)], axis=0)


# revision 41
# speedup vs baseline: 1.5156x; 1.2902x over previous
# Trainium2 Bass kernel for nn_BinaryConv (binarized VGG-ish CNN, batch 512).
#
# Strategy: pure data parallel over 8 NeuronCores (64 images each), weights
# replicated. Precision: every layer's activations are decomposed into terms
# whose matmuls accumulate exactly in fp32 PSUM:
#   - L2 input (L1 output): bf16 hi term `h` + e5m2 residual pair (t2,t3)
#     -> 9 bf16 matmuls + 9 fp8 DoubleRow matmuls per output row chunk.
#   - L3..L6 / FC inputs: 4-term fp8 split: e4m3 pair (t0,t1) + e5m2 pair
#     (t2,t3), consumed exclusively with fp8 DoubleRow matmuls (2 k-tiles
#     per instruction at 0.5 cycles/row = 4x bf16 throughput per k-tile).
# Effective mantissa ~16 bits, matching the fp32 reference closely enough
# to reproduce its (exactly one-hot) softmax output.
#
# DoubleRow pairing: vertical tap pairs (dh=0,1 at fixed dw) ride adjacent
# h-slices of the stored activation tile; the leftover dh=2 taps pair the
# two split terms of one tap (weights duplicated host-side for those rows).
# Per-layer power-of-2 output scales keep e4m3 terms below its 240 max.
# BN+bias are folded into per-channel scale/bias applied out of PSUM.

import numpy as np
import ml_dtypes

import concourse.mybir as mybir
import concourse.tile as tile
from concourse import bacc, bass
from concourse.bass_utils import run_bass_kernel_spmd

bf16 = ml_dtypes.bfloat16
e4m3 = ml_dtypes.float8_e4m3
e5m2 = ml_dtypes.float8_e5m2
F32 = mybir.dt.float32
BF16 = mybir.dt.bfloat16
F8E4 = mybir.dt.float8e4
F8E5 = mybir.dt.float8e5
Relu = mybir.ActivationFunctionType.Relu
Copy = mybir.ActivationFunctionType.Copy
Exp = mybir.ActivationFunctionType.Exp
MULT = mybir.AluOpType.mult
SUB = mybir.AluOpType.subtract
MAX = mybir.AluOpType.max
ADD = mybir.AluOpType.add
DR = mybir.MatmulPerfMode.DoubleRow

N_CORES = 8
B = 64          # images per core
SB = 16         # L1/L2 sub-batch
N_SB = 4
EPS = 1e-5

# Per-layer power-of-2 output scales (stored activation = SIG[l] * true).
# Chosen so each scaled tensor's max stays well under e4m3's 240 limit.
SIG = [2.0 ** e for e in (1, -4, -9, -14, -19, -25, -28, -33)]
# y1, p1, l3, l4(p2), l5, l6, fc1, fc2 output scales (validated in proto)

_NC_CACHE = {}


def _c(v):
    return float(np.float32(v))


def build_nc():
    if "nc" in _NC_CACHE:
        return _NC_CACHE["nc"]
    nc = bacc.Bacc(None, target_bir_lowering=False, debug=False)

    # ---------------- DRAM parameters ----------------
    xi = nc.declare_dram_parameter("xi", [N_SB, 83, 30 * 30 * SB], BF16, isOutput=False)
    w1 = nc.declare_dram_parameter("w1", [83, 128], BF16, isOutput=False)
    w2h = nc.declare_dram_parameter("w2h", [128, 9, 128], BF16, isOutput=False)
    w2e = nc.declare_dram_parameter("w2e", [128, 6, 2, 128], F8E5, isOutput=False)
    w3a = nc.declare_dram_parameter("w3a", [128, 2, 6, 2, 128], F8E4, isOutput=False)
    w3b = nc.declare_dram_parameter("w3b", [128, 2, 6, 2, 128], F8E5, isOutput=False)
    w4a = nc.declare_dram_parameter("w4a", [2, 128, 2, 6, 2, 128], F8E4, isOutput=False)
    w4b = nc.declare_dram_parameter("w4b", [2, 128, 2, 6, 2, 128], F8E5, isOutput=False)
    w5a = nc.declare_dram_parameter("w5a", [2, 128, 4, 6, 2, 128], F8E4, isOutput=False)
    w5b = nc.declare_dram_parameter("w5b", [2, 128, 4, 6, 2, 128], F8E5, isOutput=False)
    w6a = nc.declare_dram_parameter("w6a", [4, 128, 4, 5, 2, 128], F8E4, isOutput=False)
    w6b = nc.declare_dram_parameter("w6b", [4, 128, 4, 5, 2, 128], F8E5, isOutput=False)
    fw1a = nc.declare_dram_parameter("fw1a", [128, 8, 2, 2, 128], F8E4, isOutput=False)
    fw1b = nc.declare_dram_parameter("fw1b", [128, 8, 2, 2, 128], F8E5, isOutput=False)
    fw2a = nc.declare_dram_parameter("fw2a", [128, 8, 4, 2, 128], F8E4, isOutput=False)
    fw2b = nc.declare_dram_parameter("fw2b", [128, 8, 4, 2, 128], F8E5, isOutput=False)
    fw3a = nc.declare_dram_parameter("fw3a", [128, 4, 2, 16], F8E4, isOutput=False)
    fw3b = nc.declare_dram_parameter("fw3b", [128, 4, 2, 16], F8E5, isOutput=False)
    # consts columns: 0:s1 1:s2 2:t2 3-4:s3 5-6:t3 7-8:s4 9-10:t4
    # 11-14:s5 15-18:t5 19-22:s6 23-26:t6 27-34:fb1 35-42:fb2 43:fb3(rows0-9)
    consts = nc.declare_dram_parameter("consts", [128, 44], F32, isOutput=False)
    ident = nc.declare_dram_parameter("ident", [16, 16], F32, isOutput=False)
    out = nc.declare_dram_parameter("out", [B, 10], F32, isOutput=True)

    inv_f2 = _c(1.0 / SIG[7])

    with tile.TileContext(nc) as tc:
        with tc.tile_pool(name="psp", bufs=8, space="PSUM") as psp, \
             tc.tile_pool(name="p0", bufs=1) as p0:
            # ---------------- persistent tiles ----------------
            w1s = p0.tile([83, 128], BF16)
            w2hs = p0.tile([128, 9, 128], BF16)
            w2es = p0.tile([128, 6, 2, 128], F8E5)
            w3as = p0.tile([128, 2, 6, 2, 128], F8E4)
            w3bs = p0.tile([128, 2, 6, 2, 128], F8E5)
            w4as = [p0.tile([128, 2, 6, 2, 128], F8E4, name=f"w4as{i}") for i in range(2)]
            w4bs = [p0.tile([128, 2, 6, 2, 128], F8E5, name=f"w4bs{i}") for i in range(2)]
            w5as = [p0.tile([128, 4, 6, 2, 128], F8E4, name=f"w5as{i}") for i in range(2)]
            w5bs = [p0.tile([128, 4, 6, 2, 128], F8E5, name=f"w5bs{i}") for i in range(2)]
            fw1as = p0.tile([128, 8, 2, 2, 128], F8E4)
            fw1bs = p0.tile([128, 8, 2, 2, 128], F8E5)
            cs = p0.tile([128, 44], F32)
            ids = p0.tile([16, 16], F32)
            p1a = [p0.tile([128, 2, 14, 14, 32], F8E4, name=f"p1a{i}") for i in range(2)]
            p1b = [p0.tile([128, 2, 14, 14, 32], F8E5, name=f"p1b{i}") for i in range(2)]
            p2a = [p0.tile([128, 2, 5, 5, B], F8E4, name=f"p2a{i}") for i in range(2)]
            p2b = [p0.tile([128, 2, 5, 5, B], F8E5, name=f"p2b{i}") for i in range(2)]
            l5a = p0.tile([128, 2, 4, 3, 3, B], F8E4)
            l5b = p0.tile([128, 2, 4, 3, 3, B], F8E5)

            nc.sync.dma_start(out=w1s[:], in_=w1[:])
            nc.sync.dma_start(out=w2hs[:], in_=w2h[:])
            nc.sync.dma_start(out=w2es[:], in_=w2e[:])
            nc.sync.dma_start(out=cs[:], in_=consts[:])
            nc.sync.dma_start(out=ids[:], in_=ident[:])

            def load_phaseb_weights():
                nc.sync.dma_start(out=w3as[:], in_=w3a[:])
                nc.sync.dma_start(out=w3bs[:], in_=w3b[:])
                for i in range(2):
                    nc.sync.dma_start(out=w4as[i][:], in_=w4a[i])
                    nc.sync.dma_start(out=w4bs[i][:], in_=w4b[i])

            def load_phasec_weights():
                for i in range(2):
                    nc.sync.dma_start(out=w5as[i][:], in_=w5a[i])
                    nc.sync.dma_start(out=w5bs[i][:], in_=w5b[i])
                nc.sync.dma_start(out=fw1as[:], in_=fw1a[:])
                nc.sync.dma_start(out=fw1bs[:], in_=fw1b[:])

            def col(j):
                return cs[:, j:j + 1]

            class MM:
                """start/stop bookkeeping for one PSUM accumulation group."""
                def __init__(self, ps, n):
                    self.ps, self.n, self.i = ps, n, 0

                def mm(self, lhsT, rhs, dr=False):
                    nc.tensor.matmul(self.ps[:], lhsT, rhs,
                                     start=(self.i == 0), stop=(self.i == self.n - 1),
                                     perf_mode=DR if dr else None)
                    self.i += 1

            def dr9(mmo, wa_, pt, hsl, wlen):
                """9 DoubleRow matmuls covering 2 terms x 9 taps of one dtype.

                pt: [128, 2, H, W, Bd] tile; hsl: first input row; window w
                length wlen. Weights wa_: [128, 6, 2, M] pair-contiguous
                (blocks 0-2: vertical dh01 pairs per dwi; 3-5: dh2 dups)."""
                for t in range(2):
                    for dwi in range(3):
                        mmo.mm(wa_[:, dwi],
                               pt[:, t, hsl:hsl + 2, dwi:dwi + wlen, :], dr=True)
                for dwi in range(3):
                    mmo.mm(wa_[:, 3 + dwi],
                           pt[:, 0:2, hsl + 2, dwi:dwi + wlen, :], dr=True)

            def split4(pool, y32, ta0, ta1, tb0, tb1, tag, bufs=3,
                       ps=None, bias=None, scale=None, small=False):
                """4-term split of relu'd scaled fp32 y32 into e4m3 pair
                (ta0, ta1) + e5m2 pair (tb0, tb1).
                If ps is given, t0 is computed straight from PSUM (parallel
                with the caller's y32 ACT, shortening the chain). Pure casts
                ride gpsimd casting DMAs for big tiles; DVE copies for small
                ones (Pool descriptor latency dominates there)."""
                if isinstance(y32, bass.AP):
                    yap = y32
                else:
                    yap = y32[:]
                shp = [yap.partition_size(), yap.free_size()]
                y32 = None
                if ps is not None:
                    nc.scalar.activation(ta0, ps[:], Relu, bias=bias, scale=scale)
                else:
                    nc.scalar.activation(ta0, yap, Relu)
                r1 = pool.tile(shp, F32, tag=f"{tag}_r1", bufs=bufs)
                nc.vector.scalar_tensor_tensor(r1[:], yap, 1.0, ta0,
                                               op0=MULT, op1=SUB)
                if small:
                    nc.vector.tensor_copy(ta1, r1[:])
                else:
                    nc.gpsimd.dma_start(out=ta1, in_=r1[:])
                r2 = pool.tile(shp, F32, tag=f"{tag}_r2", bufs=bufs)
                nc.vector.scalar_tensor_tensor(r2[:], r1[:], 1.0, ta1,
                                               op0=MULT, op1=SUB)
                nc.scalar.activation(tb0, r2[:], Copy)
                nc.vector.scalar_tensor_tensor(tb1, r2[:], 1.0, tb0,
                                               op0=MULT, op1=SUB)

            # =============== phase A: L1, L2, pool1 (per sub-batch) ===============
            with tc.tile_pool(name="pA", bufs=1) as pA:
                l1h = pA.tile([128, 30, 30, SB], BF16, tag="l1h")
                l1e = pA.tile([128, 2, 30, 30, SB], F8E5, tag="l1e")
                prev_row = [None]

                def l1_row(sb, r):
                    ic = pA.tile([83, 30, SB], BF16, tag="ic", bufs=4, name=f"ic_{sb}_{r}")
                    nc.sync.dma_start(
                        out=ic[:], in_=xi[sb, :, r * 30 * SB:(r + 1) * 30 * SB])
                    ps = psp.tile([128, 30, SB], F32, tag="ps", name=f"ps1_{sb}_{r}")
                    nc.tensor.matmul(ps[:], w1s[:], ic[:], start=True, stop=True)
                    # s1*sig1 is folded into w1 host-side: ps is scaled+biased.
                    nc.scalar.activation(l1h[:, r], ps[:], Relu)
                    l32 = pA.tile([128, 30, SB], F32, tag="l32", bufs=2, name=f"lw_{sb}_{r}")
                    nc.vector.scalar_tensor_tensor(l32[:], ps[:], 0.0, l1h[:, r],
                                                   op0=MAX, op1=SUB)
                    nc.scalar.activation(l1e[:, 0, r], l32[:], Copy)
                    nc.vector.scalar_tensor_tensor(l1e[:, 1, r], l32[:], 1.0,
                                                   l1e[:, 0, r], op0=MULT, op1=SUB)

                def l2_row(sb, q):
                    bsl = slice(sb * SB, (sb + 1) * SB)
                    ps = psp.tile([128, 28, SB], F32, tag="ps", name=f"ps2_{sb}_{q}")
                    mmo = MM(ps, 18)
                    for dh in range(3):
                        for dw in range(3):
                            mmo.mm(w2hs[:, dh * 3 + dw, :],
                                   l1h[:, q + dh, dw:dw + 28, :])
                    dr9(mmo, w2es, l1e, q, 28)
                    y32 = pA.tile([128, 28, SB], F32, tag="y2", bufs=3,
                                  name=f"y2_{sb}_{q}")
                    nc.scalar.activation(y32[:], ps[:], Relu, bias=col(2), scale=col(1))
                    if q % 2 == 0:
                        prev_row[0] = y32
                        return
                    p = q // 2
                    rm = pA.tile([128, 28, SB], F32, tag="rm", bufs=2, name=f"rm_{sb}_{p}")
                    nc.vector.tensor_tensor(rm[:], prev_row[0][:], y32[:], op=MAX)
                    rmv = rm[:].rearrange("p (w two) b -> p w two b", two=2)
                    pw = pA.tile([128, 14, SB], F32, tag="pw", bufs=2, name=f"pw_{sb}_{p}")
                    nc.vector.tensor_tensor(pw[:], rmv[:, :, 0, :], rmv[:, :, 1, :], op=MAX)
                    hb, hsl = sb // 2, slice((sb % 2) * SB, (sb % 2 + 1) * SB)
                    sv = tc.cur_priority
                    tc.cur_priority = sv + 200000
                    split4(pA, pw,
                           p1a[hb][:, 0, p, :, hsl], p1a[hb][:, 1, p, :, hsl],
                           p1b[hb][:, 0, p, :, hsl], p1b[hb][:, 1, p, :, hsl],
                           tag="p1s", bufs=2)
                    tc.cur_priority = sv

                SKEW = 6
                for gi in range(N_SB * 30 + SKEW):
                    gq = gi - SKEW
                    if gq >= 0:
                        sb2, q = divmod(gq, 30)
                        if q < 28:
                            l2_row(sb2, q)
                    if gi < N_SB * 30:
                        sb1, r = divmod(gi, 30)
                        l1_row(sb1, r)
                    if gi == 12:
                        load_phaseb_weights()

            # ====== phase B: L3, L4, pool2 (split over batch halves) ======
            with tc.tile_pool(name="pB", bufs=1) as pB:
                for bh in range(2):
                    bsl = slice(bh * 32, (bh + 1) * 32)
                    l3a = [pB.tile([128, 2, 12, 12, 32], F8E4, tag=f"l3a{i}",
                                   name=f"l3a{i}_{bh}") for i in range(2)]
                    l3b = [pB.tile([128, 2, 12, 12, 32], F8E5, tag=f"l3b{i}",
                                   name=f"l3b{i}_{bh}") for i in range(2)]
                    # ---- L3 ----
                    for cog in range(2):
                        for r in range(12):
                            ps = psp.tile([128, 12, 32], F32, tag="ps")
                            mmo = MM(ps, 18)
                            dr9(mmo, w3as[:, cog], p1a[bh], r, 12)
                            dr9(mmo, w3bs[:, cog], p1b[bh], r, 12)
                            y32 = pB.tile([128, 12, 32], F32, tag="y3b", bufs=3,
                                          name=f"y3_{cog}_{r}_{bh}")
                            nc.scalar.activation(y32[:], ps[:], Relu,
                                                 bias=col(5 + cog), scale=col(3 + cog))
                            split4(pB, y32,
                                   l3a[cog][:, 0, r], l3a[cog][:, 1, r],
                                   l3b[cog][:, 0, r], l3b[cog][:, 1, r],
                                   tag="l3s", bufs=2,
                                   ps=ps, bias=col(5 + cog), scale=col(3 + cog))
                            if bh == 0 and cog == 1 and r == 5:
                                load_phasec_weights()
                    # ---- L4 + pool2 ----
                    for cog in range(2):
                        for p in range(5):
                            rows = []
                            for rr in range(2):
                                r = 2 * p + rr
                                ps = psp.tile([128, 10, 32], F32, tag="ps")
                                mmo = MM(ps, 36)
                                for cb in range(2):
                                    dr9(mmo, w4as[cb][:, cog], l3a[cb], r, 10)
                                    dr9(mmo, w4bs[cb][:, cog], l3b[cb], r, 10)
                                y32 = pB.tile([128, 10, 32], F32, tag="y4b", bufs=3,
                                              name=f"y4_{cog}_{bh}_{p}_{rr}")
                                nc.scalar.activation(y32[:], ps[:], Relu,
                                                     bias=col(9 + cog), scale=col(7 + cog))
                                rows.append(y32)
                            rm = pB.tile([128, 10, 32], F32, tag="rm4", bufs=2)
                            nc.vector.tensor_tensor(rm[:], rows[0][:], rows[1][:], op=MAX)
                            rmv = rm[:].rearrange("p (w two) b -> p w two b", two=2)
                            pw = pB.tile([128, 5, 32], F32, tag="pw4", bufs=2)
                            nc.vector.tensor_tensor(pw[:], rmv[:, :, 0, :],
                                                    rmv[:, :, 1, :], op=MAX)
                            sv = tc.cur_priority
                            tc.cur_priority = sv + 200000
                            split4(pB, pw,
                                   p2a[cog][:, 0, p, :, bsl], p2a[cog][:, 1, p, :, bsl],
                                   p2b[cog][:, 0, p, :, bsl], p2b[cog][:, 1, p, :, bsl],
                                   tag="p2s", bufs=2)
                            tc.cur_priority = sv

            # =============== phase C: L5, L6, FC, softmax ===============
            with tc.tile_pool(name="pC", bufs=1) as pC:
                w6as = [pC.tile([128, 4, 5, 2, 128], F8E4, name=f"w6as{i}") for i in range(4)]
                w6bs = [pC.tile([128, 4, 5, 2, 128], F8E5, name=f"w6bs{i}") for i in range(4)]
                fw2as = pC.tile([128, 8, 4, 2, 128], F8E4)
                fw2bs = pC.tile([128, 8, 4, 2, 128], F8E5)
                fw3as = pC.tile([128, 4, 2, 16], F8E4)
                fw3bs = pC.tile([128, 4, 2, 16], F8E5)
                fta = pC.tile([128, 2, 4, B], F8E4)
                ftb = pC.tile([128, 2, 4, B], F8E5)
                z1a = pC.tile([128, 2, 8, B], F8E4)
                z1b = pC.tile([128, 2, 8, B], F8E5)
                z2a = pC.tile([128, 2, 8, B], F8E4)
                z2b = pC.tile([128, 2, 8, B], F8E5)
                for i in range(4):
                    nc.sync.dma_start(out=w6as[i][:], in_=w6a[i])
                    nc.sync.dma_start(out=w6bs[i][:], in_=w6b[i])
                nc.sync.dma_start(out=fw2as[:], in_=fw2a[:])
                nc.sync.dma_start(out=fw2bs[:], in_=fw2b[:])
                nc.sync.dma_start(out=fw3as[:], in_=fw3a[:])
                nc.sync.dma_start(out=fw3bs[:], in_=fw3b[:])

                # ---- L5 ----
                for cog in range(4):
                    for h in range(3):
                        ps = psp.tile([128, 3, B], F32, tag="ps")
                        mmo = MM(ps, 36)
                        for cb in range(2):
                            dr9(mmo, w5as[cb][:, cog], p2a[cb], h, 3)
                            dr9(mmo, w5bs[cb][:, cog], p2b[cb], h, 3)
                        y32 = p0.tile([128, 3, B], F32, tag="y5c", bufs=4,
                                      name=f"y5_{cog}_{h}")
                        nc.scalar.activation(y32[:], ps[:], Relu,
                                             bias=col(15 + cog), scale=col(11 + cog))
                        split4(p0, y32,
                               l5a[:, 0, cog, h], l5a[:, 1, cog, h],
                               l5b[:, 0, cog, h], l5b[:, 1, cog, h],
                               tag="l5s", ps=ps, bias=col(15 + cog),
                               scale=col(11 + cog), small=True)

                # ---- L6 (3x3 conv on 3x3 input == dense over (ci, s)) ----
                y6s = pC.tile([128, 4, B], F32)
                for cog in range(4):
                    ps = psp.tile([128, B], F32, tag="ps")
                    mmo = MM(ps, 72)
                    for cb in range(4):
                        for (ws_, part) in ((w6as, l5a), (w6bs, l5b)):
                            pv = part[:].rearrange("p t c h w b -> p t c (h w) b")
                            for t in range(2):
                                for sp in range(4):
                                    mmo.mm(ws_[cb][:, cog, sp],
                                           pv[:, t, cb, 2 * sp:2 * sp + 2, :], dr=True)
                            mmo.mm(ws_[cb][:, cog, 4], pv[:, 0:2, cb, 8, :], dr=True)
                    nc.scalar.activation(y6s[:, cog], ps[:], Relu,
                                         bias=col(23 + cog), scale=col(19 + cog))
                split4(pC, y6s,
                       fta[:, 0], fta[:, 1], ftb[:, 0], ftb[:, 1],
                       tag="fts", small=True)

                # ---- FC1 ----
                sc_f1 = _c(SIG[6] / SIG[5])
                yf1 = pC.tile([128, 8, B], F32)
                for cog in range(8):
                    ps = psp.tile([128, B], F32, tag="ps")
                    mmo = MM(ps, 8)
                    for (ws_, pt) in ((fw1as, fta), (fw1bs, ftb)):
                        for t in range(2):
                            for kp in range(2):
                                mmo.mm(ws_[:, cog, kp],
                                       pt[:, t, 2 * kp:2 * kp + 2, :], dr=True)
                    nc.scalar.activation(yf1[:, cog], ps[:], Relu,
                                         bias=col(27 + cog), scale=sc_f1)
                for hh in range(2):
                    hs = slice(hh * 4, hh * 4 + 4)
                    split4(pC, yf1[:, hs],
                           z1a[:, 0, hs], z1a[:, 1, hs],
                           z1b[:, 0, hs], z1b[:, 1, hs],
                           tag=f"z1s{hh}", small=True)

                # ---- FC2 ----
                sc_f2 = _c(SIG[7] / SIG[6])
                yf2 = pC.tile([128, 8, B], F32)
                for cog in range(8):
                    ps = psp.tile([128, B], F32, tag="ps")
                    mmo = MM(ps, 16)
                    for (ws_, pt) in ((fw2as, z1a), (fw2bs, z1b)):
                        for t in range(2):
                            for kp in range(4):
                                mmo.mm(ws_[:, cog, kp],
                                       pt[:, t, 2 * kp:2 * kp + 2, :], dr=True)
                    nc.scalar.activation(yf2[:, cog], ps[:], Relu,
                                         bias=col(35 + cog), scale=sc_f2)
                for hh in range(2):
                    hs = slice(hh * 4, hh * 4 + 4)
                    split4(pC, yf2[:, hs],
                           z2a[:, 0, hs], z2a[:, 1, hs],
                           z2b[:, 0, hs], z2b[:, 1, hs],
                           tag=f"z2s{hh}", small=True)

                # ---- FC3 + softmax ----
                ps = psp.tile([16, B], F32, tag="ps")
                mmo = MM(ps, 16)
                for (ws_, pt) in ((fw3as, z2a), (fw3bs, z2b)):
                    for t in range(2):
                        for kp in range(4):
                            mmo.mm(ws_[:, kp], pt[:, t, 2 * kp:2 * kp + 2, :], dr=True)
                logits = pC.tile([10, B], F32)
                nc.vector.tensor_scalar(logits[:], ps[0:10, :], inv_f2, cs[0:10, 43:44],
                                        op0=MULT, op1=ADD)
                pst = psp.tile([B, 10], F32, tag="ps")
                nc.tensor.transpose(pst[:], logits[:], ids[0:10, 0:10])
                nm = pC.tile([B, 1], F32)
                nc.vector.tensor_reduce(out=nm[:], in_=pst[:], op=MAX,
                                        axis=mybir.AxisListType.X, negate=True)
                ex = pC.tile([B, 10], F32)
                sm = pC.tile([B, 1], F32)
                nc.scalar.activation(ex[:], pst[:], Exp, bias=nm[:], scale=1.0,
                                     accum_out=sm[:])
                rc = pC.tile([B, 1], F32)
                nc.vector.reciprocal(rc[:], sm[:])
                so = pC.tile([B, 10], F32)
                nc.vector.tensor_scalar_mul(so[:], ex[:], rc[:])
                nc.sync.dma_start(out=out[:], in_=so[:])

    nc.compile()
    _NC_CACHE["nc"] = nc
    return nc


# ---------------- host-side data prep ----------------

def _fold_bn(b, g, be, m, v):
    inv = (g / np.sqrt(v + EPS)).astype(np.float32)
    return inv, ((b - m) * inv + be).astype(np.float32)


def _w12(w, dt_):
    """[co, ci, 3, 3] +-1 conv weights -> [ci//128, 128, co//128, 6, 2, 128]
    pair-contiguous blocks: k in 0-2 = (dh0,dh1) vertical pair at dw=k;
    k in 3-5 = dh2 tap at dw=k-3, duplicated."""
    co, ci = w.shape[0], w.shape[1]
    ws = np.sign(w).transpose(1, 2, 3, 0)  # [ci, dh, dw, co]
    out = np.zeros((ci // 128, 128, co // 128, 6, 2, 128), dt_)
    for cb in range(ci // 128):
        blk = ws[cb * 128:(cb + 1) * 128]
        for cog in range(co // 128):
            csl = slice(cog * 128, (cog + 1) * 128)
            for dwi in range(3):
                out[cb, :, cog, dwi, 0] = blk[:, 0, dwi, csl].astype(dt_)
                out[cb, :, cog, dwi, 1] = blk[:, 1, dwi, csl].astype(dt_)
                out[cb, :, cog, 3 + dwi, 0] = blk[:, 2, dwi, csl].astype(dt_)
                out[cb, :, cog, 3 + dwi, 1] = blk[:, 2, dwi, csl].astype(dt_)
    return np.ascontiguousarray(out)


def _w10_l6(w, dt_):
    """[512, 512, 3, 3] -> [4, 128, 4, 5, 2, 128]: pair blocks 0-3 =
    (s even, s odd) pairs; block 4 = s8 duplicated."""
    ws = np.sign(w).transpose(1, 2, 3, 0).reshape(512, 9, 512)  # [ci, s, co]
    out = np.zeros((4, 128, 4, 5, 2, 128), dt_)
    for cb in range(4):
        blk = ws[cb * 128:(cb + 1) * 128]
        for cog in range(4):
            csl = slice(cog * 128, (cog + 1) * 128)
            for sp in range(4):
                out[cb, :, cog, sp, 0] = blk[:, 2 * sp, csl].astype(dt_)
                out[cb, :, cog, sp, 1] = blk[:, 2 * sp + 1, csl].astype(dt_)
            out[cb, :, cog, 4, 0] = blk[:, 8, csl].astype(dt_)
            out[cb, :, cog, 4, 1] = blk[:, 8, csl].astype(dt_)
    return np.ascontiguousarray(out)


def _wfc(w, kt, dt_):
    """[co, K] -> [128, co//128 (or 1), kt//2, 2, min(co,128)] pair-contiguous
    k-tile pairs per output block."""
    ws = np.sign(w).T  # [K, co]
    K, co = ws.shape
    kt_ = K // 128
    ncog = max(1, co // 128)
    cw = co // ncog
    out = np.zeros((128, ncog, kt_ // 2, 2, cw), dt_)
    for cog in range(ncog):
        csl = slice(cog * cw, (cog + 1) * cw)
        for kp in range(kt_ // 2):
            for j in range(2):
                kb = 2 * kp + j
                out[:, cog, kp, j] = ws[kb * 128:(kb + 1) * 128, csl].astype(dt_)
    return np.ascontiguousarray(out)


def _prep_shared(inputs):
    d = {}
    # L1: s1*sig1 folded into the weights (bf16 hi/lo rows); bias rows carry
    # t1*sig1. Rows: 27 whi (pair with x-hi), 27 whi (x-lo), 27 wlo (x-hi),
    # 2 bias. The dropped wlo*xlo term is ~2^-18 relative.
    s1f, t1f = _fold_bn(inputs["b1"], inputs["g1"], inputs["be1"],
                        inputs["m1"], inputs["v1"])
    w1c = np.sign(inputs["w1"]).transpose(1, 2, 3, 0).reshape(27, 128)
    w1sc = (w1c * (s1f * SIG[0])[None, :]).astype(np.float32)
    whi = w1sc.astype(bf16)
    wlo = (w1sc - whi.astype(np.float32)).astype(bf16)
    bias_row = (t1f * SIG[0]).astype(np.float32)
    bh = bias_row.astype(bf16)
    bl = (bias_row - bh.astype(np.float32)).astype(bf16)
    d["w1"] = np.vstack([whi, whi, wlo, bh[None, :], bl[None, :]])

    w2s = np.sign(inputs["w2"]).transpose(1, 2, 3, 0)  # [128, 3, 3, 128]
    d["w2h"] = np.ascontiguousarray(
        w2s.reshape(128, 9, 128)).astype(bf16)
    d["w2e"] = _w12(inputs["w2"], e5m2)[0, :, 0]
    d["w3a"] = _w12(inputs["w3"], e4m3)[0]
    d["w3b"] = _w12(inputs["w3"], e5m2)[0]
    d["w4a"] = _w12(inputs["w4"], e4m3)
    d["w4b"] = _w12(inputs["w4"], e5m2)
    d["w5a"] = _w12(inputs["w5"], e4m3)
    d["w5b"] = _w12(inputs["w5"], e5m2)
    d["w6a"] = _w10_l6(inputs["w6"], e4m3)
    d["w6b"] = _w10_l6(inputs["w6"], e5m2)
    d["fw1a"] = _wfc(inputs["fw1"], 4, e4m3)
    d["fw1b"] = _wfc(inputs["fw1"], 4, e5m2)
    d["fw2a"] = _wfc(inputs["fw2"], 8, e4m3)
    d["fw2b"] = _wfc(inputs["fw2"], 8, e5m2)
    fw3p = np.zeros((16, 1024), np.float32)
    fw3p[:10] = np.sign(inputs["fw3"])
    d["fw3a"] = _wfc(fw3p, 8, e4m3)[:, 0]
    d["fw3b"] = _wfc(fw3p, 8, e5m2)[:, 0]

    consts = np.zeros((128, 44), np.float32)
    # conv layers 2..6: scale = s*sig/sig_prev, bias = t*sig
    coff = [(2, 1, 2, 1), (3, 3, 5, 2), (4, 7, 9, 3), (5, 11, 15, 4),
            (6, 19, 23, 5)]
    for li, so, to, sigi in coff:
        s, t = _fold_bn(inputs[f"b{li}"], inputs[f"g{li}"], inputs[f"be{li}"],
                        inputs[f"m{li}"], inputs[f"v{li}"])
        sc = (s * (SIG[sigi] / SIG[sigi - 1])).astype(np.float32)
        tb = (t * SIG[sigi]).astype(np.float32)
        nb = len(s) // 128
        for j in range(nb):
            consts[:, so + j] = sc[j * 128:(j + 1) * 128]
            consts[:, to + j] = tb[j * 128:(j + 1) * 128]
    for j in range(8):
        consts[:, 27 + j] = inputs["fb1"][j * 128:(j + 1) * 128] * SIG[6]
        consts[:, 35 + j] = inputs["fb2"][j * 128:(j + 1) * 128] * SIG[7]
    consts[0:10, 43] = inputs["fb3"]
    d["consts"] = consts
    d["ident"] = np.eye(16, dtype=np.float32)
    return d


def _prep_x(xc):
    # xc [B, 3, 32, 32] f32 -> im2col [N_SB, 83, 30*30*SB] bf16:
    # rows 0-26 x-hi (vs whi), 27-53 x-lo (vs whi), 54-80 x-hi (vs wlo),
    # 81-82 ones (bias rows).
    x32 = xc.astype(np.float32)
    hi = x32.astype(bf16)
    lo = (x32 - hi.astype(np.float32)).astype(bf16)
    parts = []
    for p in (hi, lo, hi):
        win = np.lib.stride_tricks.sliding_window_view(p, (3, 3), axis=(2, 3))
        arr = win.transpose(1, 4, 5, 2, 3, 0).reshape(27, 30, 30, B)
        parts.append(arr)
    ones = np.ones((2, 30, 30, B), bf16)
    full = np.concatenate(parts + [ones], axis=0)  # [83, 30, 30, B]
    full = full.reshape(83, 30, 30, N_SB, SB).transpose(3, 0, 1, 2, 4)
    return np.ascontiguousarray(full).reshape(N_SB, 83, 30 * 30 * SB)


def make_in_maps(inputs):
    shared = _prep_shared(inputs)
    x = np.asarray(inputs["x"])
    in_maps = []
    for c in range(N_CORES):
        m = dict(shared)
        m["xi"] = _prep_x(x[c * B:(c + 1) * B])
        in_maps.append(m)
    return in_maps


def kernel(**inputs):
    nc = build_nc()
    in_maps = make_in_maps(inputs)
    res = run_bass_kernel_spmd(nc, in_maps, list(range(N_CORES)))
    return np.concatenate([res.results[c]["out"] for c in range(N_CORES)], axis=0)
